# revision 8
# baseline (speedup 1.0000x reference)
"""Multi-head attention (B=2, S=2048, E=1024, H=16, D=64) on 8 TRN2 cores.

Sharding: tensor-parallel over heads. Core c owns heads {2c, 2c+1}:
  - Q/K/V projections column-sharded (128 cols each per core)
  - attention for the core's 2 heads (both batches)
  - out-projection row-sharded (128 rows of Wo) -> partial [4096,1024]
  - host sums the 8 partials and adds bo.

On-chip layout (everything "transposed"):
  - host passes xT [1024, 4096] (E-major) so the contraction dim lands on
    SBUF partitions with no on-device transpose of x
  - projections produce Q^T, K^T [128, 4096] (head-dim on partitions) and
    V^T, which is PE-transposed to token-major V tiles
  - scores are computed transposed: scores^T[kk, q] so softmax's key
    reduction can ride the attn@V matmul (ones-column in V) and the
    key-padding mask folds into the exp() per-partition bias
  - attn@V emits Y^T directly (head-dim on partitions), feeding the
    row-sharded out-projection without further transposes.

Matmul inputs are bitcast to float32r: full fp32 data, full PE rate for
moving dims >= 256.
"""

import os
import numpy as np

B, S, E, H, D = 2, 2048, 1024, 16, 64
M = B * S            # 4096 tokens
P = 128              # partitions
NCORES = 8
KC = E // P          # 8 contraction chunks for projections
MCH = 512            # token chunk for projections
QCH = 512            # query chunk for attention
NEG = -1.0e30

LAST_RESULTS = None  # BassKernelResults of the most recent run (for test harness)
_PROGRAM = None


def _build_program():
    import concourse.bass as bass
    import concourse.tile as tile
    from concourse import bacc, mybir
    from concourse.masks import make_identity

    f32 = mybir.dt.float32
    f32r = mybir.dt.float32r
    r = lambda ap: ap.bitcast(f32r)

    nc = bacc.Bacc(
        "TRN2",
        target_bir_lowering=False,
        debug=False,
        enable_asserts=False,
        num_devices=NCORES,
    )

    xT_d = nc.dram_tensor("xT", (E, M), f32r, kind="ExternalInput").ap()
    wq_d = nc.dram_tensor("wq", (P, KC, P), f32r, kind="ExternalInput").ap()
    wk_d = nc.dram_tensor("wk", (P, KC, P), f32r, kind="ExternalInput").ap()
    wv_d = nc.dram_tensor("wv", (P, KC, P), f32r, kind="ExternalInput").ap()
    wo0_d = nc.dram_tensor("wo0", (D, E), f32r, kind="ExternalInput").ap()
    wo1_d = nc.dram_tensor("wo1", (D, E), f32r, kind="ExternalInput").ap()
    bq_d = nc.dram_tensor("bq", (P, 1), f32, kind="ExternalInput").ap()
    bk_d = nc.dram_tensor("bk", (P, 1), f32, kind="ExternalInput").ap()
    bv_d = nc.dram_tensor("bv", (P, 1), f32, kind="ExternalInput").ap()
    maskT_d = nc.dram_tensor("maskT", (P, B * 16), f32, kind="ExternalInput").ap()
    out_d = nc.dram_tensor("out", (M, E), f32, kind="ExternalOutput").ap()

    with tile.TileContext(nc) as tc:
        with (
            tc.tile_pool(name="consts", bufs=1) as consts,
            tc.tile_pool(name="big", bufs=1) as big,
            tc.tile_pool(name="xt_pool", bufs=4) as xt_pool,
            tc.tile_pool(name="vt_pool", bufs=2) as vt_pool,
            tc.tile_pool(name="pt_pool", bufs=6) as pt_pool,
            tc.tile_pool(name="r_pool", bufs=2) as r_pool,
            tc.tile_pool(name="out_pool", bufs=3) as out_pool,
            tc.tile_pool(name="psum_t", bufs=5, space="PSUM") as psum_t,
            tc.tile_pool(name="psum_acc", bufs=2, space="PSUM") as psum_acc,
        ):
            # ---- constants ----
            wq_sb = consts.tile([P, KC, P], f32r)
            wk_sb = consts.tile([P, KC, P], f32r)
            wv_sb = consts.tile([P, KC, P], f32r)
            wo0_sb = consts.tile([D, E], f32r)
            wo1_sb = consts.tile([D, E], f32r)
            bq_sb = consts.tile([P, 1], f32)
            bk_sb = consts.tile([P, 1], f32)
            bv_sb = consts.tile([P, 1], f32)
            mask_sb = consts.tile([P, B * 16], f32)
            ident = consts.tile([P, P], f32)
            ones_sb = consts.tile([D + 1, D], f32r)
            ones_f = consts.tile([P, D], f32)

            nc.sync.dma_start(wq_sb, wq_d)
            nc.sync.dma_start(wk_sb, wk_d)
            nc.sync.dma_start(wv_sb, wv_d)
            nc.sync.dma_start(wo0_sb, wo0_d)
            nc.sync.dma_start(wo1_sb, wo1_d)
            nc.sync.dma_start(bq_sb, bq_d)
            nc.sync.dma_start(bk_sb, bk_d)
            nc.sync.dma_start(bv_sb, bv_d)
            nc.sync.dma_start(mask_sb, maskT_d)
            make_identity(nc, ident)
            nc.vector.memset(ones_f, 1.0)
            nc.vector.tensor_copy(ones_sb, ones_f[0 : D + 1, 0:D])

            # ---- big persistent activations ----
            QT = big.tile([P, M], f32r)       # Q^T: head-dims on partitions
            KT = big.tile([P, M], f32r)
            # token-major V tiles: [tok, mt, 2*(64 cols + ones col)]
            Vtm = big.tile([P, M // P, 2 * (D + 1)], f32r)
            YT0 = big.tile([D, M], f32r)      # per-head attention output^T
            YT1 = big.tile([D, M], f32r)

            ones_col = ones_f[:, 0 : M // P].rearrange("p (a b) -> p a b", b=1)
            nc.vector.tensor_copy(Vtm[:, :, D : D + 1], ones_col)
            nc.vector.tensor_copy(Vtm[:, :, 2 * D + 1 : 2 * D + 2], ones_col)

            # ---- phase 1: projections ----
            for mc in range(M // MCH):
                msl = bass.ts(mc, MCH)
                qp = psum_t.tile([P, MCH], f32, tag="t", name="qp")
                kp = psum_t.tile([P, MCH], f32, tag="t", name="kp")
                vp = psum_t.tile([P, MCH], f32, tag="t", name="vp")
                for kc in range(KC):
                    xt = xt_pool.tile([P, MCH], f32r, tag="xt", name="xt")
                    nc.sync.dma_start(xt, xT_d[bass.ts(kc, P), msl])
                    st, sp = kc == 0, kc == KC - 1
                    nc.tensor.matmul(qp, r(wq_sb[:, kc, :]), r(xt), start=st, stop=sp)
                    nc.tensor.matmul(kp, r(wk_sb[:, kc, :]), r(xt), start=st, stop=sp)
                    nc.tensor.matmul(vp, r(wv_sb[:, kc, :]), r(xt), start=st, stop=sp)
                nc.vector.tensor_scalar_add(QT[:, msl], qp, bq_sb)
                nc.vector.tensor_scalar_add(KT[:, msl], kp, bk_sb)
                vt = vt_pool.tile([P, MCH], f32, name="vt")
                nc.vector.tensor_scalar_add(vt, vp, bv_sb)
                for j in range(MCH // P):
                    mt = mc * (MCH // P) + j
                    vtp = psum_t.tile([P, P], f32, tag="t", name="vtp")
                    nc.tensor.transpose(vtp, vt[:, bass.ts(j, P)], ident)
                    nc.vector.tensor_copy(Vtm[:, mt, 0:D], vtp[:, 0:D])
                    nc.vector.tensor_copy(Vtm[:, mt, D + 1 : 2 * D + 1], vtp[:, D : 2 * D])

            # ---- phase 2: attention (+ phase 3 out-proj per query chunk) ----
            Exp = mybir.ActivationFunctionType.Exp
            for b in range(B):
                for qc in range(S // QCH):
                    q0 = b * S + qc * QCH
                    qsl = bass.ds(q0, QCH)
                    for h in range(2):
                        d0 = D * h
                        dsl = bass.ds(d0, D)
                        YT = YT0 if h == 0 else YT1
                        av = psum_acc.tile([D + 1, QCH], f32, tag="av", name="av")
                        for t in range(S // P):
                            sc = psum_t.tile([P, QCH], f32, tag="t", name="sc")
                            k0 = b * S + t * P
                            nc.tensor.matmul(
                                sc,
                                r(KT[dsl, bass.ds(k0, P)]),
                                r(QT[dsl, qsl]),
                                start=True,
                                stop=True,
                            )
                            pt = pt_pool.tile([P, QCH], f32r, tag="pt", name="pt")
                            bt = b * 16 + t
                            nc.scalar.activation(
                                pt, sc, Exp, bias=mask_sb[:, bt : bt + 1], scale=1.0
                            )
                            nc.tensor.matmul(
                                av,
                                r(Vtm[:, b * 16 + t, h * (D + 1) : (h + 1) * (D + 1)]),
                                r(pt),
                                start=(t == 0),
                                stop=(t == S // P - 1),
                            )
                        rt = r_pool.tile([D + 1, QCH], f32r, name="rt")
                        with nc.allow_low_precision(reason="f32r softmax denom"):
                            nc.vector.reciprocal(rt[D : D + 1, :], av[D : D + 1, :])
                        rb = psum_t.tile([D, QCH], f32, tag="t", name="rb")
                        nc.tensor.matmul(
                            rb,
                            r(ones_sb[D : D + 1, :]),
                            r(rt[D : D + 1, :]),
                            start=True,
                            stop=True,
                        )
                        rbs = r_pool.tile([D, QCH], f32, tag="rbs", name="rbs")
                        nc.vector.tensor_copy(rbs, rb)
                        nc.vector.tensor_mul(YT[:, qsl], av[0:D, :], rbs)
                    # out-projection for the 4 token tiles of this query chunk
                    for j in range(QCH // P):
                        m0 = q0 + j * P
                        for ec in range(E // 512):
                            esl = bass.ts(ec, 512)
                            op = psum_t.tile([P, 512], f32, tag="t", name="op")
                            nc.tensor.matmul(
                                op,
                                r(YT0[:, bass.ds(m0, P)]),
                                r(wo0_sb[:, esl]),
                                start=True,
                                stop=False,
                            )
                            nc.tensor.matmul(
                                op,
                                r(YT1[:, bass.ds(m0, P)]),
                                r(wo1_sb[:, esl]),
                                start=False,
                                stop=True,
                            )
                            osb = out_pool.tile([P, 512], f32, name="osb")
                            nc.vector.tensor_copy(osb, op)
                            nc.sync.dma_start(out_d[bass.ds(m0, P), esl], osb)

    nc.compile()
    return nc


def kernel(x, mask, Wq, bq, Wk, bk, Wv, bv, Wo, bo):
    global LAST_RESULTS, _PROGRAM
    from concourse.bass_utils import run_bass_kernel_spmd

    if _PROGRAM is None:
        _PROGRAM = _build_program()
    nc = _PROGRAM

    x = np.asarray(x, dtype=np.float32)
    mask = np.asarray(mask)
    f32c = lambda a: np.ascontiguousarray(np.asarray(a, dtype=np.float32))

    xT = np.ascontiguousarray(x.reshape(M, E).T)                 # [E, M]
    maskf = np.where(mask, np.float32(NEG), np.float32(0.0)).astype(np.float32)
    maskT = np.ascontiguousarray(
        maskf.reshape(B, 16, P).transpose(2, 0, 1).reshape(P, B * 16)
    )
    scale = np.float32(1.0 / np.sqrt(D))

    in_maps = []
    for c in range(NCORES):
        csl = slice(P * c, P * (c + 1))
        wq_c = f32c(np.asarray(Wq)[:, csl] * scale)
        wk_c = f32c(np.asarray(Wk)[:, csl])
        wv_c = f32c(np.asarray(Wv)[:, csl])
        in_maps.append(
            {
                "xT": xT,
                "wq": np.ascontiguousarray(wq_c.reshape(KC, P, P).transpose(1, 0, 2)),
                "wk": np.ascontiguousarray(wk_c.reshape(KC, P, P).transpose(1, 0, 2)),
                "wv": np.ascontiguousarray(wv_c.reshape(KC, P, P).transpose(1, 0, 2)),
                "wo0": f32c(np.asarray(Wo)[P * c : P * c + D, :]),
                "wo1": f32c(np.asarray(Wo)[P * c + D : P * (c + 1), :]),
                "bq": f32c(np.asarray(bq)[csl] * scale).reshape(P, 1),
                "bk": f32c(np.asarray(bk)[csl]).reshape(P, 1),
                "bv": f32c(np.asarray(bv)[csl]).reshape(P, 1),
                "maskT": maskT,
            }
        )

    trace = bool(os.environ.get("KERNEL_TRACE"))
    LAST_RESULTS = run_bass_kernel_spmd(
        nc, in_maps, list(range(NCORES)), trace=trace
    )

    acc = np.zeros((M, E), dtype=np.float64)
    for res in LAST_RESULTS.results:
        acc += res["out"].astype(np.float64)
    out = (acc + np.asarray(bo, dtype=np.float64)[None, :]).astype(np.float32)
    return out.reshape(B, S, E)


# revision 10
# speedup vs baseline: 1.1227x; 1.1227x over previous
"""Multi-head attention (B=2, S=2048, E=1024, H=16, D=64) on 8 TRN2 cores.

Sharding: tensor-parallel over heads. Core c owns heads {2c, 2c+1}:
  - Q/K/V projections column-sharded (128 cols each per core)
  - attention for the core's 2 heads (both batches)
  - out-projection row-sharded (128 rows of Wo) -> partial [4096,1024]
  - host sums the 8 partials and adds bo.

On-chip layout (everything "transposed"):
  - host passes xT [1024, 4096] (E-major, fp16) so the contraction dim
    lands on SBUF partitions with no on-device transpose of x
  - projections produce Q^T, K^T [128, 4096] (head-dim on partitions) and
    V^T, which is PE-transposed to token-major V tiles
  - scores are computed transposed: scores^T[kk, q] so softmax's key
    reduction can ride the attn@V matmul (ones-column in V) and the
    key-padding mask folds into the exp() per-partition bias
  - attn@V emits Y^T directly (head-dim on partitions), feeding the
    row-sharded out-projection without further transposes.

Matmul inputs are fp16 (full PE rate, 11-bit mantissa); all accumulation
is fp32 in PSUM, and the softmax normalization chain stays fp32.
Attention loops run key-tile-outer / query-chunk-inner so consecutive PE
matmuls share the same stationary operand (amortized weight loads).
"""

import os
import numpy as np

B, S, E, H, D = 2, 2048, 1024, 16, 64
M = B * S            # 4096 tokens
P = 128              # partitions
NCORES = 8
KC = E // P          # 8 contraction chunks for projections
MCH = 512            # token chunk for projections
QCH = 512            # query chunk for attention
NQC = S // QCH       # 4 query chunks per batch
NKT = S // P         # 16 key tiles per batch
NEG = -1.0e30

LAST_RESULTS = None  # BassKernelResults of the most recent run (for test harness)
_PROGRAM = None


def _build_program():
    import concourse.bass as bass
    import concourse.tile as tile
    from concourse import bacc, mybir
    from concourse.masks import make_identity

    f32 = mybir.dt.float32
    f16 = mybir.dt.float16

    nc = bacc.Bacc(
        "TRN2",
        target_bir_lowering=False,
        debug=False,
        enable_asserts=False,
        num_devices=NCORES,
    )

    xT_d = nc.dram_tensor("xT", (E, M), f16, kind="ExternalInput").ap()
    wq_d = nc.dram_tensor("wq", (P, KC, P), f16, kind="ExternalInput").ap()
    wk_d = nc.dram_tensor("wk", (P, KC, P), f16, kind="ExternalInput").ap()
    wv_d = nc.dram_tensor("wv", (P, KC, P), f16, kind="ExternalInput").ap()
    wo0_d = nc.dram_tensor("wo0", (D, E), f16, kind="ExternalInput").ap()
    wo1_d = nc.dram_tensor("wo1", (D, E), f16, kind="ExternalInput").ap()
    bq_d = nc.dram_tensor("bq", (P, 1), f32, kind="ExternalInput").ap()
    bk_d = nc.dram_tensor("bk", (P, 1), f32, kind="ExternalInput").ap()
    bv_d = nc.dram_tensor("bv", (P, 1), f32, kind="ExternalInput").ap()
    maskT_d = nc.dram_tensor("maskT", (P, B * 16), f32, kind="ExternalInput").ap()
    out_d = nc.dram_tensor("out", (M, E), f32, kind="ExternalOutput").ap()

    with tile.TileContext(nc) as tc:
        with (
            tc.tile_pool(name="consts", bufs=1) as consts,
            tc.tile_pool(name="big", bufs=1) as big,
            tc.tile_pool(name="xt_pool", bufs=6) as xt_pool,
            tc.tile_pool(name="vt_pool", bufs=2) as vt_pool,
            tc.tile_pool(name="pt_pool", bufs=8) as pt_pool,
            tc.tile_pool(name="r_pool", bufs=2) as r_pool,
            tc.tile_pool(name="out_pool", bufs=3) as out_pool,
            tc.tile_pool(name="psum_t", bufs=4, space="PSUM") as psum_t,
            tc.tile_pool(name="psum_acc", bufs=4, space="PSUM") as psum_acc,
        ):
            # ---- constants ----
            wq_sb = consts.tile([P, KC, P], f16)
            wk_sb = consts.tile([P, KC, P], f16)
            wv_sb = consts.tile([P, KC, P], f16)
            wo0_sb = consts.tile([D, E], f16)
            wo1_sb = consts.tile([D, E], f16)
            bq_sb = consts.tile([P, 1], f32)
            bk_sb = consts.tile([P, 1], f32)
            bv_sb = consts.tile([P, 1], f32)
            mask_sb = consts.tile([P, B * 16], f32)
            ident = consts.tile([P, P], f32)
            ones_sb = consts.tile([D + 1, D], f32)
            ones_h = consts.tile([P, M // P], f16)

            nc.sync.dma_start(wq_sb, wq_d)
            nc.sync.dma_start(wk_sb, wk_d)
            nc.sync.dma_start(wv_sb, wv_d)
            nc.sync.dma_start(wo0_sb, wo0_d)
            nc.sync.dma_start(wo1_sb, wo1_d)
            nc.sync.dma_start(bq_sb, bq_d)
            nc.sync.dma_start(bk_sb, bk_d)
            nc.sync.dma_start(bv_sb, bv_d)
            nc.sync.dma_start(mask_sb, maskT_d)
            make_identity(nc, ident)
            nc.vector.memset(ones_sb, 1.0)
            nc.vector.memset(ones_h, 1.0)

            # ---- big persistent activations ----
            QT = big.tile([P, M], f16)       # Q^T: head-dims on partitions
            KT = big.tile([P, M], f16)
            # token-major V tiles: [tok, mt, 2*(64 cols + ones col)]
            Vtm = big.tile([P, M // P, 2 * (D + 1)], f16)
            YT0 = big.tile([D, M], f16)      # per-head attention output^T
            YT1 = big.tile([D, M], f16)

            ones_col = ones_h[:, 0 : M // P].rearrange("p (a b) -> p a b", b=1)
            nc.vector.tensor_copy(Vtm[:, :, D : D + 1], ones_col)
            nc.vector.tensor_copy(Vtm[:, :, 2 * D + 1 : 2 * D + 2], ones_col)

            # ---- phase 1: projections (m-chunk pairs share weight loads) ----
            for mcp in range(M // (2 * MCH)):
                psums = []
                for half in range(2):
                    mc = 2 * mcp + half
                    msl = bass.ts(mc, MCH)
                    qp = psum_t.tile([P, MCH], f32, tag="t", name="qp")
                    kp = psum_t.tile([P, MCH], f32, tag="t", name="kp")
                    vp = psum_acc.tile([P, MCH], f32, tag="av", name="vp")
                    psums.append((msl, qp, kp, vp))
                for kc in range(KC):
                    xts = []
                    for half in range(2):
                        msl = psums[half][0]
                        xt = xt_pool.tile([P, MCH], f16, tag="xt", name="xt")
                        nc.sync.dma_start(xt, xT_d[bass.ts(kc, P), msl])
                        xts.append(xt)
                    st, sp = kc == 0, kc == KC - 1
                    for wi, w_sb in ((1, wq_sb), (2, wk_sb), (3, wv_sb)):
                        for half in range(2):
                            nc.tensor.matmul(
                                psums[half][wi], w_sb[:, kc, :], xts[half],
                                start=st, stop=sp,
                            )
                for half in range(2):
                    msl, qp, kp, vp = psums[half]
                    mc = 2 * mcp + half
                    nc.vector.tensor_scalar_add(QT[:, msl], qp, bq_sb)
                    nc.vector.tensor_scalar_add(KT[:, msl], kp, bk_sb)
                    vt = vt_pool.tile([P, MCH], f32, name="vt")
                    nc.vector.tensor_scalar_add(vt, vp, bv_sb)
                    for j in range(MCH // P):
                        mt = mc * (MCH // P) + j
                        vtp = psum_acc.tile([P, P], f32, tag="av", name="vtp")
                        nc.tensor.transpose(vtp, vt[:, bass.ts(j, P)], ident)
                        nc.vector.tensor_copy(Vtm[:, mt, 0:D], vtp[:, 0:D])
                        nc.vector.tensor_copy(
                            Vtm[:, mt, D + 1 : 2 * D + 1], vtp[:, D : 2 * D]
                        )

            # ---- phase 2: attention (+ out-proj per batch/head pair) ----
            Exp = mybir.ActivationFunctionType.Exp
            for b in range(B):
                for h in range(2):
                    d0 = D * h
                    dsl = bass.ds(d0, D)
                    YT = YT0 if h == 0 else YT1
                    avs = [
                        psum_acc.tile([D + 1, QCH], f32, tag="av", name="av")
                        for _ in range(NQC)
                    ]
                    # key-tile outer: consecutive matmuls share stationary lhsT
                    for t in range(NKT):
                        ksl = bass.ds(b * S + t * P, P)
                        lhs_k = KT[dsl, ksl]
                        pts = []
                        for qc in range(NQC):
                            qsl = bass.ds(b * S + qc * QCH, QCH)
                            sc = psum_t.tile([P, QCH], f32, tag="t", name="sc")
                            nc.tensor.matmul(sc, lhs_k, QT[dsl, qsl], start=True, stop=True)
                            pt = pt_pool.tile([P, QCH], f16, tag="pt", name="pt")
                            bt = b * 16 + t
                            nc.scalar.activation(
                                pt, sc, Exp, bias=mask_sb[:, bt : bt + 1], scale=1.0
                            )
                            pts.append(pt)
                        lhs_v = Vtm[:, b * 16 + t, h * (D + 1) : (h + 1) * (D + 1)]
                        for qc in range(NQC):
                            nc.tensor.matmul(
                                avs[qc], lhs_v, pts[qc],
                                start=(t == 0), stop=(t == NKT - 1),
                            )
                    for qc in range(NQC):
                        qsl = bass.ds(b * S + qc * QCH, QCH)
                        av = avs[qc]
                        rt = r_pool.tile([D + 1, QCH], f32, name="rt")
                        nc.vector.reciprocal(rt[D : D + 1, :], av[D : D + 1, :])
                        rb = psum_t.tile([D, QCH], f32, tag="t", name="rb")
                        nc.tensor.matmul(
                            rb, ones_sb[D : D + 1, :], rt[D : D + 1, :],
                            start=True, stop=True,
                        )
                        rbs = r_pool.tile([D, QCH], f32, tag="rbs", name="rbs")
                        nc.vector.tensor_copy(rbs, rb)
                        nc.vector.tensor_mul(YT[:, qsl], av[0:D, :], rbs)
                # out-projection for this batch (whole 2048 tokens)
                for j in range(S // P):
                    m0 = b * S + j * P
                    for ec in range(E // 512):
                        esl = bass.ts(ec, 512)
                        op = psum_t.tile([P, 512], f32, tag="t", name="op")
                        nc.tensor.matmul(
                            op, YT0[:, bass.ds(m0, P)], wo0_sb[:, esl],
                            start=True, stop=False,
                        )
                        nc.tensor.matmul(
                            op, YT1[:, bass.ds(m0, P)], wo1_sb[:, esl],
                            start=False, stop=True,
                        )
                        osb = out_pool.tile([P, 512], f32, name="osb")
                        nc.vector.tensor_copy(osb, op)
                        nc.sync.dma_start(out_d[bass.ds(m0, P), esl], osb)

    nc.compile()
    return nc


def kernel(x, mask, Wq, bq, Wk, bk, Wv, bv, Wo, bo):
    global LAST_RESULTS, _PROGRAM
    import ml_dtypes
    from concourse.bass_utils import run_bass_kernel_spmd

    if _PROGRAM is None:
        _PROGRAM = _build_program()
    nc = _PROGRAM

    f16 = np.float16
    x = np.asarray(x, dtype=np.float32)
    mask = np.asarray(mask)
    f32c = lambda a: np.ascontiguousarray(np.asarray(a, dtype=np.float32))

    xT = np.ascontiguousarray(x.reshape(M, E).T.astype(f16))     # [E, M]
    maskf = np.where(mask, np.float32(NEG), np.float32(0.0)).astype(np.float32)
    maskT = np.ascontiguousarray(
        maskf.reshape(B, 16, P).transpose(2, 0, 1).reshape(P, B * 16)
    )
    scale = np.float32(1.0 / np.sqrt(D))

    in_maps = []
    for c in range(NCORES):
        csl = slice(P * c, P * (c + 1))
        wq_c = (np.asarray(Wq, dtype=np.float32)[:, csl] * scale).astype(f16)
        wk_c = np.asarray(Wk, dtype=np.float32)[:, csl].astype(f16)
        wv_c = np.asarray(Wv, dtype=np.float32)[:, csl].astype(f16)
        in_maps.append(
            {
                "xT": xT,
                "wq": np.ascontiguousarray(wq_c.reshape(KC, P, P).transpose(1, 0, 2)),
                "wk": np.ascontiguousarray(wk_c.reshape(KC, P, P).transpose(1, 0, 2)),
                "wv": np.ascontiguousarray(wv_c.reshape(KC, P, P).transpose(1, 0, 2)),
                "wo0": np.ascontiguousarray(
                    np.asarray(Wo, dtype=np.float32)[P * c : P * c + D, :].astype(f16)
                ),
                "wo1": np.ascontiguousarray(
                    np.asarray(Wo, dtype=np.float32)[P * c + D : P * (c + 1), :].astype(f16)
                ),
                "bq": f32c(np.asarray(bq)[csl] * scale).reshape(P, 1),
                "bk": f32c(np.asarray(bk)[csl]).reshape(P, 1),
                "bv": f32c(np.asarray(bv)[csl]).reshape(P, 1),
                "maskT": maskT,
            }
        )

    trace = bool(os.environ.get("KERNEL_TRACE"))
    LAST_RESULTS = run_bass_kernel_spmd(
        nc, in_maps, list(range(NCORES)), trace=trace
    )

    acc = np.zeros((M, E), dtype=np.float64)
    for res in LAST_RESULTS.results:
        acc += res["out"].astype(np.float64)
    out = (acc + np.asarray(bo, dtype=np.float64)[None, :]).astype(np.float32)
    return out.reshape(B, S, E)


# revision 13
# speedup vs baseline: 1.2262x; 1.0921x over previous
"""Multi-head attention (B=2, S=2048, E=1024, H=16, D=64) on 8 TRN2 cores.

Sharding: tensor-parallel over heads. Core c owns heads {2c, 2c+1}:
  - Q/K/V projections column-sharded (128 cols each per core)
  - attention for the core's 2 heads (both batches)
  - out-projection row-sharded (128 rows of Wo) -> partial [4096,1024]
  - host sums the 8 partials and adds bo.

On-chip layout (everything "transposed"):
  - host passes xT [1024, 4096] (E-major, fp16) so the contraction dim
    lands on SBUF partitions with no on-device transpose of x
  - projections produce Q^T, K^T [128, 4096] (head-dim on partitions) and
    V^T, which is PE-transposed to token-major V tiles
  - scores are computed transposed: scores^T[kk, q] so softmax's key
    reduction can ride the attn@V matmul (ones-column in V) and the
    key-padding mask folds into the exp() per-partition bias
  - attn@V emits Y^T directly (head-dim on partitions), feeding the
    row-sharded out-projection without further transposes.

Matmul inputs are fp16 (full PE rate, 11-bit mantissa); all accumulation
is fp32 in PSUM, and the softmax normalization chain stays fp32.
Attention loops run key-tile-outer / query-chunk-inner so consecutive PE
matmuls share the same stationary operand (amortized weight loads).
"""

import os
import numpy as np

B, S, E, H, D = 2, 2048, 1024, 16, 64
M = B * S            # 4096 tokens
P = 128              # partitions
NCORES = 8
KC = E // P          # 8 contraction chunks for projections
MCH = 512            # token chunk for projections
QCH = 512            # query chunk for attention
NQC = S // QCH       # 4 query chunks per batch
NKT = S // P         # 16 key tiles per batch
NEG = -1.0e30

LAST_RESULTS = None  # BassKernelResults of the most recent run (for test harness)
_PROGRAM = None


def _build_program():
    import concourse.bass as bass
    import concourse.tile as tile
    from concourse import bacc, mybir
    from concourse.masks import make_identity

    f32 = mybir.dt.float32
    f16 = mybir.dt.float16

    nc = bacc.Bacc(
        "TRN2",
        target_bir_lowering=False,
        debug=False,
        enable_asserts=False,
        num_devices=NCORES,
    )

    xT_d = nc.dram_tensor("xT", (E, M), f16, kind="ExternalInput").ap()
    wq_d = nc.dram_tensor("wq", (P, KC, P), f16, kind="ExternalInput").ap()
    wk_d = nc.dram_tensor("wk", (P, KC, P), f16, kind="ExternalInput").ap()
    wv_d = nc.dram_tensor("wv", (P, KC, P), f16, kind="ExternalInput").ap()
    wo0_d = nc.dram_tensor("wo0", (D, E), f16, kind="ExternalInput").ap()
    wo1_d = nc.dram_tensor("wo1", (D, E), f16, kind="ExternalInput").ap()
    bq_d = nc.dram_tensor("bq", (P, 1), f32, kind="ExternalInput").ap()
    bk_d = nc.dram_tensor("bk", (P, 1), f32, kind="ExternalInput").ap()
    bv_d = nc.dram_tensor("bv", (P, 1), f32, kind="ExternalInput").ap()
    maskT_d = nc.dram_tensor("maskT", (P, B * 16), f32, kind="ExternalInput").ap()
    out_d = nc.dram_tensor("out", (M, E), f32, kind="ExternalOutput").ap()

    with tile.TileContext(nc) as tc:
        with (
            tc.tile_pool(name="consts", bufs=1) as consts,
            tc.tile_pool(name="big", bufs=1) as big,
            tc.tile_pool(name="xt_pool", bufs=6) as xt_pool,
            tc.tile_pool(name="vt_pool", bufs=2) as vt_pool,
            tc.tile_pool(name="pt_pool", bufs=8) as pt_pool,
            tc.tile_pool(name="r_pool", bufs=2) as r_pool,
            tc.tile_pool(name="out_pool", bufs=3) as out_pool,
            tc.tile_pool(name="psum_t", bufs=4, space="PSUM") as psum_t,
            tc.tile_pool(name="psum_acc", bufs=4, space="PSUM") as psum_acc,
        ):
            # ---- constants ----
            wq_sb = consts.tile([P, KC, P], f16)
            wk_sb = consts.tile([P, KC, P], f16)
            wv_sb = consts.tile([P, KC, P], f16)
            wo0_sb = consts.tile([D, E], f16)
            wo1_sb = consts.tile([D, E], f16)
            bq_sb = consts.tile([P, 1], f32)
            bk_sb = consts.tile([P, 1], f32)
            bv_sb = consts.tile([P, 1], f32)
            mask_sb = consts.tile([P, B * 16], f32)
            ident = consts.tile([P, P], f32)
            ones_sb = consts.tile([D + 1, D], f32)
            ones_h = consts.tile([P, M // P], f16)

            nc.sync.dma_start(wq_sb, wq_d)
            nc.sync.dma_start(wk_sb, wk_d)
            nc.sync.dma_start(wv_sb, wv_d)
            nc.sync.dma_start(wo0_sb, wo0_d)
            nc.sync.dma_start(wo1_sb, wo1_d)
            nc.sync.dma_start(bq_sb, bq_d)
            nc.sync.dma_start(bk_sb, bk_d)
            nc.sync.dma_start(bv_sb, bv_d)
            nc.sync.dma_start(mask_sb, maskT_d)
            make_identity(nc, ident)
            nc.vector.memset(ones_sb, 1.0)
            nc.vector.memset(ones_h, 1.0)

            # ---- big persistent activations ----
            QT = big.tile([P, M], f16)       # Q^T: head-dims on partitions
            KT = big.tile([P, M], f16)
            # token-major V tiles: [tok, mt, 2*(64 cols + ones col)]
            Vtm = big.tile([P, M // P, 2 * (D + 1)], f16)
            YT0 = big.tile([D, M], f16)      # per-head attention output^T
            YT1 = big.tile([D, M], f16)

            ones_col = ones_h[:, 0 : M // P].rearrange("p (a b) -> p a b", b=1)
            nc.vector.tensor_copy(Vtm[:, :, D : D + 1], ones_col)
            nc.vector.tensor_copy(Vtm[:, :, 2 * D + 1 : 2 * D + 2], ones_col)

            # ---- phase 1: projections (m-chunk pairs share weight loads) ----
            for mcp in range(M // (2 * MCH)):
                psums = []
                for half in range(2):
                    mc = 2 * mcp + half
                    msl = bass.ts(mc, MCH)
                    qp = psum_t.tile([P, MCH], f32, tag="t", name="qp")
                    kp = psum_t.tile([P, MCH], f32, tag="t", name="kp")
                    vp = psum_acc.tile([P, MCH], f32, tag="av", name="vp")
                    psums.append((msl, qp, kp, vp))
                for kc in range(KC):
                    xts = []
                    for half in range(2):
                        msl = psums[half][0]
                        xt = xt_pool.tile([P, MCH], f16, tag="xt", name="xt")
                        nc.sync.dma_start(xt, xT_d[bass.ts(kc, P), msl])
                        xts.append(xt)
                    st, sp = kc == 0, kc == KC - 1
                    for wi, w_sb in ((1, wq_sb), (2, wk_sb), (3, wv_sb)):
                        for half in range(2):
                            nc.tensor.matmul(
                                psums[half][wi], w_sb[:, kc, :], xts[half],
                                start=st, stop=sp,
                            )
                for half in range(2):
                    msl, qp, kp, vp = psums[half]
                    mc = 2 * mcp + half
                    nc.vector.tensor_scalar_add(QT[:, msl], qp, bq_sb)
                    nc.vector.tensor_scalar_add(KT[:, msl], kp, bk_sb)
                    vt = vt_pool.tile([P, MCH], f32, name="vt")
                    nc.vector.tensor_scalar_add(vt, vp, bv_sb)
                    for j in range(MCH // P):
                        mt = mc * (MCH // P) + j
                        vtp = psum_acc.tile([P, P], f32, tag="av", name="vtp")
                        nc.tensor.transpose(vtp, vt[:, bass.ts(j, P)], ident)
                        nc.vector.tensor_copy(Vtm[:, mt, 0:D], vtp[:, 0:D])
                        nc.vector.tensor_copy(
                            Vtm[:, mt, D + 1 : 2 * D + 1], vtp[:, D : 2 * D]
                        )

            # ---- phase 2: attention (+ deferred normalization / out-proj) ----
            # Normalization for group i is emitted after group i+1's matmul
            # stream so the PE never stalls on the DVE reciprocal.
            Exp = mybir.ActivationFunctionType.Exp

            def emit_norm(b, h, av_sbs):
                YT = YT0 if h == 0 else YT1
                for qc in range(NQC):
                    qsl = bass.ds(b * S + qc * QCH, QCH)
                    av_sb = av_sbs[qc]
                    rt = r_pool.tile([D + 1, QCH], f32, name="rt")
                    nc.vector.reciprocal(rt[D : D + 1, :], av_sb[D : D + 1, :])
                    rb = psum_t.tile([D, QCH], f32, tag="t", name="rb")
                    nc.tensor.matmul(
                        rb, ones_sb[D : D + 1, :], rt[D : D + 1, :],
                        start=True, stop=True,
                    )
                    nc.vector.tensor_mul(YT[:, qsl], av_sb[0:D, :], rb)

            def emit_outproj(b):
                for j in range(S // P):
                    m0 = b * S + j * P
                    for ec in range(E // 512):
                        esl = bass.ts(ec, 512)
                        op = psum_t.tile([P, 512], f32, tag="t", name="op")
                        nc.tensor.matmul(
                            op, YT0[:, bass.ds(m0, P)], wo0_sb[:, esl],
                            start=True, stop=False,
                        )
                        nc.tensor.matmul(
                            op, YT1[:, bass.ds(m0, P)], wo1_sb[:, esl],
                            start=False, stop=True,
                        )
                        osb = out_pool.tile([P, 512], f32, name="osb")
                        nc.vector.tensor_copy(osb, op)
                        nc.sync.dma_start(out_d[bass.ds(m0, P), esl], osb)

            pending = []
            for b in range(B):
                for h in range(2):
                    d0 = D * h
                    dsl = bass.ds(d0, D)
                    avs = [
                        psum_acc.tile([D + 1, QCH], f32, tag="av", name="av")
                        for _ in range(NQC)
                    ]
                    # key-tile outer: consecutive matmuls share stationary lhsT
                    for t in range(NKT):
                        ksl = bass.ds(b * S + t * P, P)
                        lhs_k = KT[dsl, ksl]
                        pts = []
                        for qc in range(NQC):
                            qsl = bass.ds(b * S + qc * QCH, QCH)
                            sc = psum_t.tile([P, QCH], f32, tag="t", name="sc")
                            nc.tensor.matmul(sc, lhs_k, QT[dsl, qsl], start=True, stop=True)
                            pt = pt_pool.tile([P, QCH], f16, tag="pt", name="pt")
                            bt = b * 16 + t
                            nc.scalar.activation(
                                pt, sc, Exp, bias=mask_sb[:, bt : bt + 1], scale=1.0
                            )
                            pts.append(pt)
                        lhs_v = Vtm[:, b * 16 + t, h * (D + 1) : (h + 1) * (D + 1)]
                        for qc in range(NQC):
                            nc.tensor.matmul(
                                avs[qc], lhs_v, pts[qc],
                                start=(t == 0), stop=(t == NKT - 1),
                            )
                    # stage accumulators to SBUF, freeing the PSUM bank group
                    av_sbs = []
                    for qc in range(NQC):
                        av_sb = r_pool.tile([D + 1, QCH], f32, tag="avsb", bufs=8, name="avsb")
                        nc.vector.tensor_copy(av_sb, avs[qc])
                        av_sbs.append(av_sb)
                    pending.append((b, h, av_sbs))
                    if len(pending) > 1:
                        emit_norm(*pending.pop(0))
                    if len(pending) == 1 and b == 1 and h == 0:
                        emit_outproj(0)
            while pending:
                emit_norm(*pending.pop(0))
            emit_outproj(1)

    nc.compile()
    return nc


def kernel(x, mask, Wq, bq, Wk, bk, Wv, bv, Wo, bo):
    global LAST_RESULTS, _PROGRAM
    import ml_dtypes
    from concourse.bass_utils import run_bass_kernel_spmd

    if _PROGRAM is None:
        _PROGRAM = _build_program()
    nc = _PROGRAM

    f16 = np.float16
    x = np.asarray(x, dtype=np.float32)
    mask = np.asarray(mask)
    f32c = lambda a: np.ascontiguousarray(np.asarray(a, dtype=np.float32))

    xT = np.ascontiguousarray(x.reshape(M, E).T.astype(f16))     # [E, M]
    maskf = np.where(mask, np.float32(NEG), np.float32(0.0)).astype(np.float32)
    maskT = np.ascontiguousarray(
        maskf.reshape(B, 16, P).transpose(2, 0, 1).reshape(P, B * 16)
    )
    scale = np.float32(1.0 / np.sqrt(D))

    in_maps = []
    for c in range(NCORES):
        csl = slice(P * c, P * (c + 1))
        wq_c = (np.asarray(Wq, dtype=np.float32)[:, csl] * scale).astype(f16)
        wk_c = np.asarray(Wk, dtype=np.float32)[:, csl].astype(f16)
        wv_c = np.asarray(Wv, dtype=np.float32)[:, csl].astype(f16)
        in_maps.append(
            {
                "xT": xT,
                "wq": np.ascontiguousarray(wq_c.reshape(KC, P, P).transpose(1, 0, 2)),
                "wk": np.ascontiguousarray(wk_c.reshape(KC, P, P).transpose(1, 0, 2)),
                "wv": np.ascontiguousarray(wv_c.reshape(KC, P, P).transpose(1, 0, 2)),
                "wo0": np.ascontiguousarray(
                    np.asarray(Wo, dtype=np.float32)[P * c : P * c + D, :].astype(f16)
                ),
                "wo1": np.ascontiguousarray(
                    np.asarray(Wo, dtype=np.float32)[P * c + D : P * (c + 1), :].astype(f16)
                ),
                "bq": f32c(np.asarray(bq)[csl] * scale).reshape(P, 1),
                "bk": f32c(np.asarray(bk)[csl]).reshape(P, 1),
                "bv": f32c(np.asarray(bv)[csl]).reshape(P, 1),
                "maskT": maskT,
            }
        )

    trace = bool(os.environ.get("KERNEL_TRACE"))
    LAST_RESULTS = run_bass_kernel_spmd(
        nc, in_maps, list(range(NCORES)), trace=trace
    )

    acc = np.zeros((M, E), dtype=np.float64)
    for res in LAST_RESULTS.results:
        acc += res["out"].astype(np.float64)
    out = (acc + np.asarray(bo, dtype=np.float64)[None, :]).astype(np.float32)
    return out.reshape(B, S, E)


# revision 18
# speedup vs baseline: 1.4209x; 1.1588x over previous
"""Multi-head attention (B=2, S=2048, E=1024, H=16, D=64) on 8 TRN2 cores.

Sharding: tensor-parallel over heads. Core c owns heads {2c, 2c+1}:
  - Q/K/V projections column-sharded (128 cols each per core)
  - attention for the core's 2 heads (both batches)
  - out-projection row-sharded (128 rows of Wo) -> partial [4096,1024]
  - host sums the 8 partials and adds bo.

On-chip layout (everything "transposed"):
  - host passes xT [1024, 4096] (E-major, fp16) so the contraction dim
    lands on SBUF partitions with no on-device transpose of x
  - projections produce Q^T, K^T [128, 4096] (head-dim on partitions) and
    V^T, which is PE-transposed to token-major V tiles
  - scores are computed transposed: scores^T[kk, q] so softmax's key
    reduction can ride the attn@V matmul (ones-column in V) and the
    key-padding mask folds into the exp() per-partition bias
  - attn@V emits Y^T directly (head-dim on partitions), feeding the
    row-sharded out-projection without further transposes.

Perf notes:
  - matmul inputs fp16 (full PE rate); accumulation fp32 in PSUM;
    softmax normalization chain fp32
  - TRN2's PE p-state controller halves the clock when the engine idles,
    so the attention loop is shaped to keep PE saturated: exp() batched
    [128,1024] on ACT (faster per step than the PE work it feeds),
    normalization runs entirely on DVE+DMA (stride-0 partition-broadcast
    DMA instead of a ones-matmul), and each group's normalization is
    emitted one pass late so the PE never waits on the DVE reciprocal
  - consecutive PE matmuls share their stationary operand (weight-load
    amortization): key-tile-outer loops, paired m-chunks in projections
"""

import os
import numpy as np

B, S, E, H, D = 2, 2048, 1024, 16, 64
M = B * S            # 4096 tokens
P = 128              # partitions
NCORES = 8
KC = E // P          # 8 contraction chunks for projections
MCH = 512            # token chunk for projections
QCH = 512            # query chunk for attention
NQC = S // QCH       # 4 query chunks per batch
NKT = S // P         # 16 key tiles per batch
NEG = -1.0e30

LAST_RESULTS = None  # BassKernelResults of the most recent run (for test harness)
_PROGRAM = None


def _build_program():
    import concourse.bass as bass
    import concourse.tile as tile
    from concourse import bacc, mybir
    from concourse.masks import make_identity

    f32 = mybir.dt.float32
    f16 = mybir.dt.float16

    nc = bacc.Bacc(
        "TRN2",
        target_bir_lowering=False,
        debug=False,
        enable_asserts=False,
        num_devices=NCORES,
    )

    xT_d = nc.dram_tensor("xT", (E, M), f16, kind="ExternalInput").ap()
    wq_d = nc.dram_tensor("wq", (P, KC, P), f16, kind="ExternalInput").ap()
    wk_d = nc.dram_tensor("wk", (P, KC, P), f16, kind="ExternalInput").ap()
    wv_d = nc.dram_tensor("wv", (P, KC, P), f16, kind="ExternalInput").ap()
    wo0_d = nc.dram_tensor("wo0", (D, E), f16, kind="ExternalInput").ap()
    wo1_d = nc.dram_tensor("wo1", (D, E), f16, kind="ExternalInput").ap()
    bq_d = nc.dram_tensor("bq", (P, 1), f32, kind="ExternalInput").ap()
    bk_d = nc.dram_tensor("bk", (P, 1), f32, kind="ExternalInput").ap()
    bv_d = nc.dram_tensor("bv", (P, 1), f32, kind="ExternalInput").ap()
    maskT_d = nc.dram_tensor("maskT", (P, B * 16), f32, kind="ExternalInput").ap()
    out_d = nc.dram_tensor("out", (M, E), f32, kind="ExternalOutput").ap()
    rsc_d = nc.dram_tensor("rscratch", (16, QCH), f32, kind="Internal").ap()

    with tile.TileContext(nc) as tc:
        with (
            tc.tile_pool(name="consts", bufs=1) as consts,
            tc.tile_pool(name="big", bufs=1) as big,
            tc.tile_pool(name="xt_pool", bufs=6) as xt_pool,
            tc.tile_pool(name="vt_pool", bufs=2) as vt_pool,
            tc.tile_pool(name="pt_pool", bufs=6) as pt_pool,
            tc.tile_pool(name="r_pool", bufs=2) as r_pool,
            tc.tile_pool(name="out_pool", bufs=4) as out_pool,
        ):
            # ---- constants ----
            wq_sb = consts.tile([P, KC, P], f16)
            wk_sb = consts.tile([P, KC, P], f16)
            wv_sb = consts.tile([P, KC, P], f16)
            wo0_sb = consts.tile([D, E], f16)
            wo1_sb = consts.tile([D, E], f16)
            bq_sb = consts.tile([P, 1], f32)
            bk_sb = consts.tile([P, 1], f32)
            bv_sb = consts.tile([P, 1], f32)
            mask_sb = consts.tile([P, B * 16], f32)
            ident = consts.tile([P, P], f32)
            ones_h = consts.tile([P, M // P], f16)

            nc.sync.dma_start(wq_sb, wq_d)
            nc.sync.dma_start(wk_sb, wk_d)
            nc.sync.dma_start(wv_sb, wv_d)
            nc.sync.dma_start(wo0_sb, wo0_d)
            nc.sync.dma_start(wo1_sb, wo1_d)
            nc.sync.dma_start(bq_sb, bq_d)
            nc.sync.dma_start(bk_sb, bk_d)
            nc.sync.dma_start(bv_sb, bv_d)
            nc.sync.dma_start(mask_sb, maskT_d)
            make_identity(nc, ident)
            nc.vector.memset(ones_h, 1.0)

            # ---- big persistent activations ----
            QT = big.tile([P, M], f16)       # Q^T: head-dims on partitions
            KT = big.tile([P, M], f16)
            # token-major V tiles: [tok, mt, 2*(64 cols + ones col)]
            Vtm = big.tile([P, M // P, 2 * (D + 1)], f16)
            YT0 = big.tile([D, M], f16)      # per-head attention output^T
            YT1 = big.tile([D, M], f16)

            ones_col = ones_h[:, 0 : M // P].rearrange("p (a b) -> p a b", b=1)
            nc.vector.tensor_copy(Vtm[:, :, D : D + 1], ones_col)
            nc.vector.tensor_copy(Vtm[:, :, 2 * D + 1 : 2 * D + 2], ones_col)

            # ---- phase 1: projections (m-chunk pairs share weight loads) ----
            with tc.tile_pool(name="psum_p1", bufs=6, space="PSUM") as psum_p1:
                for mcp in range(M // (2 * MCH)):
                    psums = []
                    for half in range(2):
                        mc = 2 * mcp + half
                        msl = bass.ts(mc, MCH)
                        qp = psum_p1.tile([P, MCH], f32, tag="p1", name="qp")
                        kp = psum_p1.tile([P, MCH], f32, tag="p1", name="kp")
                        vp = psum_p1.tile([P, MCH], f32, tag="p1", name="vp")
                        psums.append((msl, qp, kp, vp))
                    for kc in range(KC):
                        xts = []
                        for half in range(2):
                            msl = psums[half][0]
                            xt = xt_pool.tile([P, MCH], f16, tag="xt", name="xt")
                            nc.sync.dma_start(xt, xT_d[bass.ts(kc, P), msl])
                            xts.append(xt)
                        st, sp = kc == 0, kc == KC - 1
                        for wi, w_sb in ((1, wq_sb), (2, wk_sb), (3, wv_sb)):
                            for half in range(2):
                                nc.tensor.matmul(
                                    psums[half][wi], w_sb[:, kc, :], xts[half],
                                    start=st, stop=sp,
                                )
                    for half in range(2):
                        msl, qp, kp, vp = psums[half]
                        mc = 2 * mcp + half
                        nc.vector.tensor_scalar_add(QT[:, msl], qp, bq_sb)
                        nc.vector.tensor_scalar_add(KT[:, msl], kp, bk_sb)
                        vt = vt_pool.tile([P, MCH], f32, name="vt")
                        nc.vector.tensor_scalar_add(vt, vp, bv_sb)
                        for j in range(MCH // P):
                            mt = mc * (MCH // P) + j
                            vtp = psum_p1.tile([P, P], f32, tag="vtp", bufs=2, name="vtp")
                            nc.tensor.transpose(vtp, vt[:, bass.ts(j, P)], ident)
                            nc.vector.tensor_copy(Vtm[:, mt, 0:D], vtp[:, 0:D])
                            nc.vector.tensor_copy(
                                Vtm[:, mt, D + 1 : 2 * D + 1], vtp[:, D : 2 * D]
                            )

            # ---- phase 2: attention, deferred normalization, out-proj ----
            Exp = mybir.ActivationFunctionType.Exp
            with (
                tc.tile_pool(name="psum_sc", bufs=2, space="PSUM") as psum_sc,
                tc.tile_pool(name="psum_av", bufs=2, space="PSUM") as psum_av,
                tc.tile_pool(name="psum_op", bufs=2, space="PSUM") as psum_op,
            ):
                copy_flip = [0]
                norm_idx = [0]

                def psum_to_sbuf(dst, src):
                    # alternate ACT/DVE so neither engine becomes the tail
                    if copy_flip[0] % 2 == 0:
                        nc.vector.tensor_copy(dst, src)
                    else:
                        nc.scalar.copy(dst, src)
                    copy_flip[0] += 1

                def emit_norm(b, h, pr, av_sbs):
                    YT = YT0 if h == 0 else YT1
                    for qi in range(2):
                        qc = 2 * pr + qi
                        qsl = bass.ds(b * S + qc * QCH, QCH)
                        av_sb = av_sbs[qi]
                        rt = r_pool.tile([D + 1, QCH], f32, name="rt")
                        nc.vector.reciprocal(rt[D : D + 1, :], av_sb[D : D + 1, :])
                        # partition-broadcast [1,512] -> [64,512] via DRAM bounce
                        # (SBUF-source DMAs cannot have a zero partition step)
                        ni = norm_idx[0]
                        norm_idx[0] += 1
                        nc.sync.dma_start(rsc_d[ni, :], rt[D : D + 1, :])
                        rbs = r_pool.tile([D, QCH], f32, tag="rbs", bufs=4, name="rbs")
                        src = rsc_d[ni : ni + 1, :]
                        src_b = bass.AP(
                            tensor=src.tensor,
                            offset=src.offset,
                            ap=[[0, D]] + [list(x) for x in src.ap[1:]],
                        )
                        nc.sync.dma_start(rbs, src_b)
                        nc.vector.tensor_mul(YT[:, qsl], av_sb[0:D, :], rbs)

                def emit_outproj(b, jlo, jhi):
                    for j in range(jlo, jhi):
                        m0 = b * S + j * P
                        for ec in range(E // 512):
                            esl = bass.ts(ec, 512)
                            op = psum_op.tile([P, 512], f32, tag="op", name="op")
                            nc.tensor.matmul(
                                op, YT0[:, bass.ds(m0, P)], wo0_sb[:, esl],
                                start=True, stop=False,
                            )
                            nc.tensor.matmul(
                                op, YT1[:, bass.ds(m0, P)], wo1_sb[:, esl],
                                start=False, stop=True,
                            )
                            osb = out_pool.tile([P, 512], f32, name="osb")
                            psum_to_sbuf(osb, op)
                            nc.sync.dma_start(out_d[bass.ds(m0, P), esl], osb)

                passes = [(b, h, pr) for b in range(B) for h in range(2) for pr in range(2)]
                pending = []
                for pi, (b, h, pr) in enumerate(passes):
                    dsl = bass.ds(D * h, D)
                    avs = [
                        psum_av.tile([D + 1, QCH], f32, tag="av", name="av")
                        for _ in range(2)
                    ]
                    for t in range(NKT):
                        ksl = bass.ds(b * S + t * P, P)
                        lhs_k = KT[dsl, ksl]
                        sc2 = psum_sc.tile([P, 2 * QCH], f32, tag="sc", name="sc2")
                        for qi in range(2):
                            qc = 2 * pr + qi
                            qsl = bass.ds(b * S + qc * QCH, QCH)
                            nc.tensor.matmul(
                                sc2[:, bass.ts(qi, QCH)], lhs_k, QT[dsl, qsl],
                                start=True, stop=True,
                            )
                        pt = pt_pool.tile([P, 2 * QCH], f16, tag="pt", name="pt")
                        bt = b * 16 + t
                        nc.scalar.activation(
                            pt, sc2, Exp, bias=mask_sb[:, bt : bt + 1], scale=1.0
                        )
                        lhs_v = Vtm[:, bt, h * (D + 1) : (h + 1) * (D + 1)]
                        for qi in range(2):
                            nc.tensor.matmul(
                                avs[qi], lhs_v, pt[:, bass.ts(qi, QCH)],
                                start=(t == 0), stop=(t == NKT - 1),
                            )
                    # stage accumulators to SBUF, freeing the PSUM banks
                    av_sbs = []
                    for qi in range(2):
                        av_sb = r_pool.tile(
                            [D + 1, QCH], f32, tag="avsb", bufs=6, name="avsb"
                        )
                        psum_to_sbuf(av_sb, avs[qi])
                        av_sbs.append(av_sb)
                    pending.append((b, h, pr, av_sbs))
                    if len(pending) > 1:
                        emit_norm(*pending.pop(0))
                    if pi == 4:
                        # norms for all of batch 0 have been emitted
                        emit_outproj(0, 0, S // P)
                # tail: batch-1 pair-0 out-proj covers the last norm's latency
                emit_outproj(1, 0, S // (2 * P))
                emit_norm(*pending.pop(0))            # (1,1,1)
                emit_outproj(1, S // (2 * P), S // P)

    nc.compile()
    return nc


def kernel(x, mask, Wq, bq, Wk, bk, Wv, bv, Wo, bo):
    global LAST_RESULTS, _PROGRAM
    from concourse.bass_utils import run_bass_kernel_spmd

    if _PROGRAM is None:
        _PROGRAM = _build_program()
    nc = _PROGRAM

    f16 = np.float16
    x = np.asarray(x, dtype=np.float32)
    mask = np.asarray(mask)
    f32c = lambda a: np.ascontiguousarray(np.asarray(a, dtype=np.float32))

    xT = np.ascontiguousarray(x.reshape(M, E).T.astype(f16))     # [E, M]
    maskf = np.where(mask, np.float32(NEG), np.float32(0.0)).astype(np.float32)
    maskT = np.ascontiguousarray(
        maskf.reshape(B, 16, P).transpose(2, 0, 1).reshape(P, B * 16)
    )
    scale = np.float32(1.0 / np.sqrt(D))

    in_maps = []
    for c in range(NCORES):
        csl = slice(P * c, P * (c + 1))
        wq_c = (np.asarray(Wq, dtype=np.float32)[:, csl] * scale).astype(f16)
        wk_c = np.asarray(Wk, dtype=np.float32)[:, csl].astype(f16)
        wv_c = np.asarray(Wv, dtype=np.float32)[:, csl].astype(f16)
        in_maps.append(
            {
                "xT": xT,
                "wq": np.ascontiguousarray(wq_c.reshape(KC, P, P).transpose(1, 0, 2)),
                "wk": np.ascontiguousarray(wk_c.reshape(KC, P, P).transpose(1, 0, 2)),
                "wv": np.ascontiguousarray(wv_c.reshape(KC, P, P).transpose(1, 0, 2)),
                "wo0": np.ascontiguousarray(
                    np.asarray(Wo, dtype=np.float32)[P * c : P * c + D, :].astype(f16)
                ),
                "wo1": np.ascontiguousarray(
                    np.asarray(Wo, dtype=np.float32)[P * c + D : P * (c + 1), :].astype(f16)
                ),
                "bq": f32c(np.asarray(bq)[csl] * scale).reshape(P, 1),
                "bk": f32c(np.asarray(bk)[csl]).reshape(P, 1),
                "bv": f32c(np.asarray(bv)[csl]).reshape(P, 1),
                "maskT": maskT,
            }
        )

    trace = bool(os.environ.get("KERNEL_TRACE"))
    LAST_RESULTS = run_bass_kernel_spmd(
        nc, in_maps, list(range(NCORES)), trace=trace
    )

    acc = np.zeros((M, E), dtype=np.float64)
    for res in LAST_RESULTS.results:
        acc += res["out"].astype(np.float64)
    out = (acc + np.asarray(bo, dtype=np.float64)[None, :]).astype(np.float32)
    return out.reshape(B, S, E)


# revision 23
# speedup vs baseline: 1.4607x; 1.0280x over previous
"""Multi-head attention (B=2, S=2048, E=1024, H=16, D=64) on 8 TRN2 cores.

Sharding: tensor-parallel over heads. Core c owns heads {2c, 2c+1}:
  - Q/K/V projections column-sharded (128 cols each per core)
  - attention for the core's 2 heads (both batches)
  - out-projection row-sharded (128 rows of Wo) -> partial [4096,1024]
  - host sums the 8 partials and adds bo.

On-chip layout (everything "transposed"):
  - host passes xT [1024, 4096] (E-major, fp16) so the contraction dim
    lands on SBUF partitions with no on-device transpose of x
  - projections produce Q^T, K^T [128, 4096] (head-dim on partitions) and
    V^T, which is PE-transposed to token-major V tiles
  - scores are computed transposed: scores^T[kk, q] so softmax's key
    reduction can ride the attn@V matmul (ones-column in V) and the
    key-padding mask folds into the exp() per-partition bias
  - attn@V emits Y^T directly (head-dim on partitions), feeding the
    row-sharded out-projection without further transposes.

Perf notes:
  - matmul inputs fp16 (full PE rate); accumulation fp32 in PSUM;
    softmax normalization chain fp32
  - TRN2's PE p-state controller halves the clock when the engine idles,
    so the attention loop is shaped to keep PE saturated: exp() batched
    [128,1024] on ACT (faster per step than the PE work it feeds),
    normalization runs entirely on DVE+DMA (stride-0 partition-broadcast
    DMA instead of a ones-matmul), and each group's normalization is
    emitted one pass late so the PE never waits on the DVE reciprocal
  - consecutive PE matmuls share their stationary operand (weight-load
    amortization): key-tile-outer loops, paired m-chunks in projections
"""

import os
import numpy as np

B, S, E, H, D = 2, 2048, 1024, 16, 64
M = B * S            # 4096 tokens
P = 128              # partitions
NCORES = 8
KC = E // P          # 8 contraction chunks for projections
MCH = 512            # token chunk for projections
QCH = 512            # query chunk for attention
NQC = S // QCH       # 4 query chunks per batch
NKT = S // P         # 16 key tiles per batch
NEG = -1.0e30

LAST_RESULTS = None  # BassKernelResults of the most recent run (for test harness)
_PROGRAM = None


def _build_program():
    import concourse.bass as bass
    import concourse.tile as tile
    from concourse import bacc, mybir
    from concourse.masks import make_identity

    f32 = mybir.dt.float32
    f16 = mybir.dt.float16

    nc = bacc.Bacc(
        "TRN2",
        target_bir_lowering=False,
        debug=False,
        enable_asserts=False,
        num_devices=NCORES,
    )

    xT_d = nc.dram_tensor("xT", (E, M), f16, kind="ExternalInput").ap()
    wq_d = nc.dram_tensor("wq", (P, KC, P), f16, kind="ExternalInput").ap()
    wk_d = nc.dram_tensor("wk", (P, KC, P), f16, kind="ExternalInput").ap()
    wv_d = nc.dram_tensor("wv", (P, KC, P), f16, kind="ExternalInput").ap()
    wo0_d = nc.dram_tensor("wo0", (D, E), f16, kind="ExternalInput").ap()
    wo1_d = nc.dram_tensor("wo1", (D, E), f16, kind="ExternalInput").ap()
    bq_d = nc.dram_tensor("bq", (P, 1), f32, kind="ExternalInput").ap()
    bk_d = nc.dram_tensor("bk", (P, 1), f32, kind="ExternalInput").ap()
    bv_d = nc.dram_tensor("bv", (P, 1), f32, kind="ExternalInput").ap()
    maskT_d = nc.dram_tensor("maskT", (P, B * 16), f32, kind="ExternalInput").ap()
    out_d = nc.dram_tensor("out", (M, E), f32, kind="ExternalOutput").ap()
    rsc_d = nc.dram_tensor("rscratch", (16, QCH), f32, kind="Internal").ap()

    with tile.TileContext(nc) as tc:
        with (
            tc.tile_pool(name="consts", bufs=1) as consts,
            tc.tile_pool(name="big", bufs=1) as big,
            tc.tile_pool(name="xt_pool", bufs=6) as xt_pool,
            tc.tile_pool(name="vt_pool", bufs=2) as vt_pool,
            tc.tile_pool(name="pt_pool", bufs=6) as pt_pool,
            tc.tile_pool(name="r_pool", bufs=2) as r_pool,
            tc.tile_pool(name="out_pool", bufs=4) as out_pool,
        ):
            # ---- constants ----
            wq_sb = consts.tile([P, KC, P], f16)
            wk_sb = consts.tile([P, KC, P], f16)
            wv_sb = consts.tile([P, KC, P], f16)
            wo0_sb = consts.tile([D, E], f16)
            wo1_sb = consts.tile([D, E], f16)
            bq_sb = consts.tile([P, 1], f32)
            bk_sb = consts.tile([P, 1], f32)
            bv_sb = consts.tile([P, 1], f32)
            mask_sb = consts.tile([P, B * 16], f32)
            ident = consts.tile([P, P], f32)
            ones_h = consts.tile([P, M // P], f16)

            nc.sync.dma_start(wq_sb, wq_d)
            nc.sync.dma_start(wk_sb, wk_d)
            nc.sync.dma_start(wv_sb, wv_d)
            nc.sync.dma_start(wo0_sb, wo0_d)
            nc.sync.dma_start(wo1_sb, wo1_d)
            nc.sync.dma_start(bq_sb, bq_d)
            nc.sync.dma_start(bk_sb, bk_d)
            nc.sync.dma_start(bv_sb, bv_d)
            nc.sync.dma_start(mask_sb, maskT_d)
            make_identity(nc, ident)
            nc.vector.memset(ones_h, 1.0)

            # ---- big persistent activations ----
            QT = big.tile([P, M], f16)       # Q^T: head-dims on partitions
            KT = big.tile([P, M], f16)
            # token-major V tiles: [tok, mt, 2*(64 cols + ones col)]
            Vtm = big.tile([P, M // P, 2 * (D + 1)], f16)
            YT0 = big.tile([D, M], f16)      # per-head attention output^T
            YT1 = big.tile([D, M], f16)

            ones_col = ones_h[:, 0 : M // P].rearrange("p (a b) -> p a b", b=1)
            nc.vector.tensor_copy(Vtm[:, :, D : D + 1], ones_col)
            nc.vector.tensor_copy(Vtm[:, :, 2 * D + 1 : 2 * D + 2], ones_col)

            # ---- phase 1: projections (m-chunk pairs share weight loads) ----
            with tc.tile_pool(name="psum_p1", bufs=6, space="PSUM") as psum_p1:
                for mcp in range(M // (2 * MCH)):
                    psums = []
                    for half in range(2):
                        mc = 2 * mcp + half
                        msl = bass.ts(mc, MCH)
                        qp = psum_p1.tile([P, MCH], f32, tag="p1", name="qp")
                        kp = psum_p1.tile([P, MCH], f32, tag="p1", name="kp")
                        vp = psum_p1.tile([P, MCH], f32, tag="p1", name="vp")
                        psums.append((msl, qp, kp, vp))
                    for kc in range(KC):
                        xts = []
                        for half in range(2):
                            msl = psums[half][0]
                            xt = xt_pool.tile([P, MCH], f16, tag="xt", name="xt")
                            nc.sync.dma_start(xt, xT_d[bass.ts(kc, P), msl])
                            xts.append(xt)
                        st, sp = kc == 0, kc == KC - 1
                        for wi, w_sb in ((1, wq_sb), (2, wk_sb), (3, wv_sb)):
                            for half in range(2):
                                nc.tensor.matmul(
                                    psums[half][wi], w_sb[:, kc, :], xts[half],
                                    start=st, stop=sp,
                                )
                    for half in range(2):
                        msl, qp, kp, vp = psums[half]
                        mc = 2 * mcp + half
                        nc.vector.tensor_scalar_add(QT[:, msl], qp, bq_sb)
                        nc.vector.tensor_scalar_add(KT[:, msl], kp, bk_sb)
                        vt = vt_pool.tile([P, MCH], f32, name="vt")
                        nc.vector.tensor_scalar_add(vt, vp, bv_sb)
                        for j in range(MCH // P):
                            mt = mc * (MCH // P) + j
                            vtp = psum_p1.tile([P, P], f32, tag="vtp", bufs=2, name="vtp")
                            nc.tensor.transpose(vtp, vt[:, bass.ts(j, P)], ident)
                            nc.vector.tensor_copy(Vtm[:, mt, 0:D], vtp[:, 0:D])
                            nc.vector.tensor_copy(
                                Vtm[:, mt, D + 1 : 2 * D + 1], vtp[:, D : 2 * D]
                            )

            # ---- phase 2: attention, deferred normalization, out-proj ----
            Exp = mybir.ActivationFunctionType.Exp
            with (
                tc.tile_pool(name="psum_sc", bufs=2, space="PSUM") as psum_sc,
                tc.tile_pool(name="psum_av", bufs=2, space="PSUM") as psum_av,
                tc.tile_pool(name="psum_op", bufs=2, space="PSUM") as psum_op,
            ):
                norm_idx = [0]

                def psum_to_sbuf(dst, src):
                    # DVE only: ACT must stay a pure-exp stream, or its stalls
                    # starve the PE and drop the p-state
                    nc.vector.tensor_copy(dst, src)

                def emit_norm(b, h, pr, av_sbs):
                    YT = YT0 if h == 0 else YT1
                    for qi in range(2):
                        qc = 2 * pr + qi
                        qsl = bass.ds(b * S + qc * QCH, QCH)
                        av_sb = av_sbs[qi]
                        rt = r_pool.tile([D + 1, QCH], f32, name="rt")
                        nc.vector.reciprocal(rt[D : D + 1, :], av_sb[D : D + 1, :])
                        # partition-broadcast [1,512] -> [64,512] via DRAM bounce
                        # (SBUF-source DMAs cannot have a zero partition step)
                        ni = norm_idx[0]
                        norm_idx[0] += 1
                        nc.sync.dma_start(rsc_d[ni, :], rt[D : D + 1, :])
                        rbs = r_pool.tile([D, QCH], f32, tag="rbs", bufs=4, name="rbs")
                        src = rsc_d[ni : ni + 1, :]
                        src_b = bass.AP(
                            tensor=src.tensor,
                            offset=src.offset,
                            ap=[[0, D]] + [list(x) for x in src.ap[1:]],
                        )
                        nc.sync.dma_start(rbs, src_b)
                        nc.vector.tensor_mul(YT[:, qsl], av_sb[0:D, :], rbs)

                def emit_outproj_tile(b, j):
                    m0 = b * S + j * P
                    for ec in range(E // 512):
                        esl = bass.ts(ec, 512)
                        op = psum_op.tile([P, 512], f32, tag="op", name="op")
                        nc.tensor.matmul(
                            op, YT0[:, bass.ds(m0, P)], wo0_sb[:, esl],
                            start=True, stop=False,
                        )
                        nc.tensor.matmul(
                            op, YT1[:, bass.ds(m0, P)], wo1_sb[:, esl],
                            start=False, stop=True,
                        )
                        osb = out_pool.tile([P, 512], f32, name="osb")
                        psum_to_sbuf(osb, op)
                        nc.sync.dma_start(out_d[bass.ds(m0, P), esl], osb)

                def emit_outproj(b, jlo, jhi):
                    for j in range(jlo, jhi):
                        emit_outproj_tile(b, j)

                passes = [(b, h, pr) for b in range(B) for h in range(2) for pr in range(2)]
                pending = []
                filler = []  # (b, j) out-proj tiles interleaved as PE work
                for pi, (b, h, pr) in enumerate(passes):
                    dsl = bass.ds(D * h, D)
                    avs = [
                        psum_av.tile([D + 1, QCH], f32, tag="av", name="av")
                        for _ in range(2)
                    ]
                    for t in range(NKT):
                        ksl = bass.ds(b * S + t * P, P)
                        lhs_k = KT[dsl, ksl]
                        sc2 = psum_sc.tile([P, 2 * QCH], f32, tag="sc", name="sc2")
                        for qi in range(2):
                            qc = 2 * pr + qi
                            qsl = bass.ds(b * S + qc * QCH, QCH)
                            nc.tensor.matmul(
                                sc2[:, bass.ts(qi, QCH)], lhs_k, QT[dsl, qsl],
                                start=True, stop=True,
                            )
                        pt = pt_pool.tile([P, 2 * QCH], f16, tag="pt", name="pt")
                        bt = b * 16 + t
                        nc.scalar.activation(
                            pt, sc2, Exp, bias=mask_sb[:, bt : bt + 1], scale=1.0
                        )
                        lhs_v = Vtm[:, bt, h * (D + 1) : (h + 1) * (D + 1)]
                        for qi in range(2):
                            nc.tensor.matmul(
                                avs[qi], lhs_v, pt[:, bass.ts(qi, QCH)],
                                start=(t == 0), stop=(t == NKT - 1),
                            )
                        # interleave ready out-proj tiles as PE filler so the
                        # engine stays saturated through ACT hiccups
                        if filler and t % 3 == 2:
                            emit_outproj_tile(*filler.pop(0))
                    # stage accumulators to SBUF, freeing the PSUM banks
                    av_sbs = []
                    for qi in range(2):
                        av_sb = r_pool.tile(
                            [D + 1, QCH], f32, tag="avsb", bufs=6, name="avsb"
                        )
                        psum_to_sbuf(av_sb, avs[qi])
                        av_sbs.append(av_sb)
                    pending.append((b, h, pr, av_sbs))
                    if len(pending) > 1:
                        emit_norm(*pending.pop(0))
                    if pi == 4:
                        # norms for all of batch 0 have been emitted
                        filler.extend((0, j) for j in range(S // P))
                # drain leftover batch-0 filler, then the batch-1 tail
                for item in filler:
                    emit_outproj_tile(*item)
                emit_outproj(1, 0, S // (2 * P))
                emit_norm(*pending.pop(0))            # (1,1,1)
                emit_outproj(1, S // (2 * P), S // P)

    nc.compile()
    return nc


def kernel(x, mask, Wq, bq, Wk, bk, Wv, bv, Wo, bo):
    global LAST_RESULTS, _PROGRAM
    from concourse.bass_utils import run_bass_kernel_spmd

    if _PROGRAM is None:
        _PROGRAM = _build_program()
    nc = _PROGRAM

    f16 = np.float16
    x = np.asarray(x, dtype=np.float32)
    mask = np.asarray(mask)
    f32c = lambda a: np.ascontiguousarray(np.asarray(a, dtype=np.float32))

    xT = np.ascontiguousarray(x.reshape(M, E).T.astype(f16))     # [E, M]
    maskf = np.where(mask, np.float32(NEG), np.float32(0.0)).astype(np.float32)
    maskT = np.ascontiguousarray(
        maskf.reshape(B, 16, P).transpose(2, 0, 1).reshape(P, B * 16)
    )
    scale = np.float32(1.0 / np.sqrt(D))

    in_maps = []
    for c in range(NCORES):
        csl = slice(P * c, P * (c + 1))
        wq_c = (np.asarray(Wq, dtype=np.float32)[:, csl] * scale).astype(f16)
        wk_c = np.asarray(Wk, dtype=np.float32)[:, csl].astype(f16)
        wv_c = np.asarray(Wv, dtype=np.float32)[:, csl].astype(f16)
        in_maps.append(
            {
                "xT": xT,
                "wq": np.ascontiguousarray(wq_c.reshape(KC, P, P).transpose(1, 0, 2)),
                "wk": np.ascontiguousarray(wk_c.reshape(KC, P, P).transpose(1, 0, 2)),
                "wv": np.ascontiguousarray(wv_c.reshape(KC, P, P).transpose(1, 0, 2)),
                "wo0": np.ascontiguousarray(
                    np.asarray(Wo, dtype=np.float32)[P * c : P * c + D, :].astype(f16)
                ),
                "wo1": np.ascontiguousarray(
                    np.asarray(Wo, dtype=np.float32)[P * c + D : P * (c + 1), :].astype(f16)
                ),
                "bq": f32c(np.asarray(bq)[csl] * scale).reshape(P, 1),
                "bk": f32c(np.asarray(bk)[csl]).reshape(P, 1),
                "bv": f32c(np.asarray(bv)[csl]).reshape(P, 1),
                "maskT": maskT,
            }
        )

    trace = bool(os.environ.get("KERNEL_TRACE"))
    LAST_RESULTS = run_bass_kernel_spmd(
        nc, in_maps, list(range(NCORES)), trace=trace
    )

    acc = np.zeros((M, E), dtype=np.float64)
    for res in LAST_RESULTS.results:
        acc += res["out"].astype(np.float64)
    out = (acc + np.asarray(bo, dtype=np.float64)[None, :]).astype(np.float32)
    return out.reshape(B, S, E)


# revision 24
# speedup vs baseline: 1.7185x; 1.1765x over previous
"""Multi-head attention (B=2, S=2048, E=1024, H=16, D=64) on 8 TRN2 cores.

Sharding: tensor-parallel over heads. Core c owns heads {2c, 2c+1}:
  - Q/K/V projections column-sharded (128 cols each per core)
  - attention for the core's 2 heads (both batches)
  - out-projection row-sharded (128 rows of Wo) -> partial [4096,1024]
  - host sums the 8 partials and adds bo.

On-chip layout (everything "transposed"):
  - host passes xT [1024, 4096] (E-major, fp16) so the contraction dim
    lands on SBUF partitions with no on-device transpose of x
  - projections produce Q^T, K^T [128, 4096] (head-dim on partitions) and
    V^T, which is PE-transposed to token-major V tiles
  - scores are computed transposed: scores^T[kk, q] so softmax's key
    reduction can ride the attn@V matmul (ones-column in V) and the
    key-padding mask folds into the exp() per-partition bias
  - attn@V emits Y^T directly (head-dim on partitions), feeding the
    row-sharded out-projection without further transposes.

Perf notes:
  - matmul inputs fp16 (full PE rate); accumulation fp32 in PSUM;
    softmax normalization chain fp32
  - TRN2's PE p-state controller halves the clock when the engine idles,
    so the attention loop is shaped to keep PE saturated: exp() batched
    [128,1024] on ACT (faster per step than the PE work it feeds),
    normalization runs entirely on DVE+DMA (stride-0 partition-broadcast
    DMA instead of a ones-matmul), and each group's normalization is
    emitted one pass late so the PE never waits on the DVE reciprocal
  - consecutive PE matmuls share their stationary operand (weight-load
    amortization): key-tile-outer loops, paired m-chunks in projections
"""

import os
import numpy as np

B, S, E, H, D = 2, 2048, 1024, 16, 64
M = B * S            # 4096 tokens
P = 128              # partitions
NCORES = 8
KC = E // P          # 8 contraction chunks for projections
MCH = 512            # token chunk for projections
QCH = 512            # query chunk for attention
NQC = S // QCH       # 4 query chunks per batch
NKT = S // P         # 16 key tiles per batch
NEG = -1.0e30

LAST_RESULTS = None  # BassKernelResults of the most recent run (for test harness)
_PROGRAM = None


def _build_program():
    import concourse.bass as bass
    import concourse.tile as tile
    from concourse import bacc, mybir
    from concourse.masks import make_identity

    f32 = mybir.dt.float32
    f16 = mybir.dt.float16

    nc = bacc.Bacc(
        "TRN2",
        target_bir_lowering=False,
        debug=False,
        enable_asserts=False,
        num_devices=NCORES,
    )

    xT_d = nc.dram_tensor("xT", (E, M), f16, kind="ExternalInput").ap()
    wq_d = nc.dram_tensor("wq", (P, KC, P), f16, kind="ExternalInput").ap()
    wk_d = nc.dram_tensor("wk", (P, KC, P), f16, kind="ExternalInput").ap()
    wv_d = nc.dram_tensor("wv", (P, KC, P), f16, kind="ExternalInput").ap()
    wo0_d = nc.dram_tensor("wo0", (D, E), f16, kind="ExternalInput").ap()
    wo1_d = nc.dram_tensor("wo1", (D, E), f16, kind="ExternalInput").ap()
    bq_d = nc.dram_tensor("bq", (P, 1), f32, kind="ExternalInput").ap()
    bk_d = nc.dram_tensor("bk", (P, 1), f32, kind="ExternalInput").ap()
    bv_d = nc.dram_tensor("bv", (P, 1), f32, kind="ExternalInput").ap()
    maskT_d = nc.dram_tensor("maskT", (P, B * 16), f32, kind="ExternalInput").ap()
    out_d = nc.dram_tensor("out", (M, E), f32, kind="ExternalOutput").ap()
    rsc_d = nc.dram_tensor("rscratch", (16, QCH), f32, kind="Internal").ap()

    with tile.TileContext(nc) as tc:
        with (
            tc.tile_pool(name="consts", bufs=1) as consts,
            tc.tile_pool(name="big", bufs=1) as big,
            tc.tile_pool(name="xt_pool", bufs=6) as xt_pool,
            tc.tile_pool(name="vt_pool", bufs=2) as vt_pool,
            tc.tile_pool(name="pt_pool", bufs=6) as pt_pool,
            tc.tile_pool(name="r_pool", bufs=2) as r_pool,
            tc.tile_pool(name="out_pool", bufs=4) as out_pool,
        ):
            # ---- constants ----
            wq_sb = consts.tile([P, KC, P], f16)
            wk_sb = consts.tile([P, KC, P], f16)
            wv_sb = consts.tile([P, KC, P], f16)
            wo0_sb = consts.tile([D, E], f16)
            wo1_sb = consts.tile([D, E], f16)
            bq_sb = consts.tile([P, 1], f32)
            bk_sb = consts.tile([P, 1], f32)
            bv_sb = consts.tile([P, 1], f32)
            mask_sb = consts.tile([P, B * 16], f32)
            ident = consts.tile([P, P], f32)
            ones_h = consts.tile([P, M // P], f16)

            nc.sync.dma_start(wq_sb, wq_d)
            nc.sync.dma_start(wk_sb, wk_d)
            nc.sync.dma_start(wv_sb, wv_d)
            nc.sync.dma_start(wo0_sb, wo0_d)
            nc.sync.dma_start(wo1_sb, wo1_d)
            nc.sync.dma_start(bq_sb, bq_d)
            nc.sync.dma_start(bk_sb, bk_d)
            nc.sync.dma_start(bv_sb, bv_d)
            nc.sync.dma_start(mask_sb, maskT_d)
            make_identity(nc, ident)
            nc.vector.memset(ones_h, 1.0)

            # ---- big persistent activations ----
            QT = big.tile([P, M], f16)       # Q^T: head-dims on partitions
            KT = big.tile([P, M], f16)
            # token-major V tiles: [tok, mt, 2*(64 cols + ones col)]
            Vtm = big.tile([P, M // P, 2 * (D + 1)], f16)
            YT0 = big.tile([D, M], f16)      # per-head attention output^T
            YT1 = big.tile([D, M], f16)

            ones_col = ones_h[:, 0 : M // P].rearrange("p (a b) -> p a b", b=1)
            nc.vector.tensor_copy(Vtm[:, :, D : D + 1], ones_col)
            nc.vector.tensor_copy(Vtm[:, :, 2 * D + 1 : 2 * D + 2], ones_col)

            # ---- phase 1: projections (m-chunk pairs share weight loads) ----
            with tc.tile_pool(name="psum_p1", bufs=6, space="PSUM") as psum_p1:
                for mcp in range(M // (2 * MCH)):
                    psums = []
                    for half in range(2):
                        mc = 2 * mcp + half
                        msl = bass.ts(mc, MCH)
                        qp = psum_p1.tile([P, MCH], f32, tag="p1", name="qp")
                        kp = psum_p1.tile([P, MCH], f32, tag="p1", name="kp")
                        vp = psum_p1.tile([P, MCH], f32, tag="p1", name="vp")
                        psums.append((msl, qp, kp, vp))
                    for kc in range(KC):
                        xts = []
                        for half in range(2):
                            msl = psums[half][0]
                            xt = xt_pool.tile([P, MCH], f16, tag="xt", name="xt")
                            nc.sync.dma_start(xt, xT_d[bass.ts(kc, P), msl])
                            xts.append(xt)
                        st, sp = kc == 0, kc == KC - 1
                        for wi, w_sb in ((1, wq_sb), (2, wk_sb), (3, wv_sb)):
                            for half in range(2):
                                nc.tensor.matmul(
                                    psums[half][wi], w_sb[:, kc, :], xts[half],
                                    start=st, stop=sp,
                                )
                    for half in range(2):
                        msl, qp, kp, vp = psums[half]
                        mc = 2 * mcp + half
                        nc.vector.tensor_scalar_add(QT[:, msl], qp, bq_sb)
                        nc.vector.tensor_scalar_add(KT[:, msl], kp, bk_sb)
                        vt = vt_pool.tile([P, MCH], f32, name="vt")
                        nc.vector.tensor_scalar_add(vt, vp, bv_sb)
                        for j in range(MCH // P):
                            mt = mc * (MCH // P) + j
                            vtp = psum_p1.tile([P, P], f32, tag="vtp", bufs=2, name="vtp")
                            nc.tensor.transpose(vtp, vt[:, bass.ts(j, P)], ident)
                            nc.vector.tensor_copy(Vtm[:, mt, 0:D], vtp[:, 0:D])
                            nc.vector.tensor_copy(
                                Vtm[:, mt, D + 1 : 2 * D + 1], vtp[:, D : 2 * D]
                            )

            # ---- phase 2: attention, deferred normalization, out-proj ----
            Exp = mybir.ActivationFunctionType.Exp
            with (
                tc.tile_pool(name="psum_sc", bufs=2, space="PSUM") as psum_sc,
                tc.tile_pool(name="psum_av", bufs=2, space="PSUM") as psum_av,
                tc.tile_pool(name="psum_op", bufs=2, space="PSUM") as psum_op,
            ):
                norm_idx = [0]

                def psum_to_sbuf(dst, src):
                    # DVE only: ACT must stay a pure-exp stream, or its stalls
                    # starve the PE and drop the p-state
                    nc.vector.tensor_copy(dst, src)

                def emit_norm(b, h, pr, av_sbs):
                    YT = YT0 if h == 0 else YT1
                    for qi in range(2):
                        qc = 2 * pr + qi
                        qsl = bass.ds(b * S + qc * QCH, QCH)
                        av_sb = av_sbs[qi]
                        # partition-broadcast the raw sums [1,512] -> [64,512]
                        # via DRAM bounce (SBUF-source DMAs cannot have a zero
                        # partition step), then reciprocal at base partition 0
                        # (custom-DVE approx ops misbehave at base 64)
                        ni = norm_idx[0]
                        norm_idx[0] += 1
                        nc.sync.dma_start(rsc_d[ni, :], av_sb[D : D + 1, :])
                        sb = r_pool.tile([D, QCH], f32, tag="sb", bufs=4, name="sb")
                        src = rsc_d[ni : ni + 1, :]
                        src_b = bass.AP(
                            tensor=src.tensor,
                            offset=src.offset,
                            ap=[[0, D]] + [list(x) for x in src.ap[1:]],
                        )
                        nc.sync.dma_start(sb, src_b)
                        rbs = r_pool.tile([D, QCH], f32, tag="rbs", bufs=4, name="rbs")
                        rsc2 = r_pool.tile([D, QCH], f32, tag="rsc2", name="rsc2")
                        nc.vector.reciprocal_approx_accurate(rbs, sb, rsc2)
                        nc.vector.tensor_mul(YT[:, qsl], av_sb[0:D, :], rbs)

                def emit_outproj_tile(b, j):
                    m0 = b * S + j * P
                    for ec in range(E // 512):
                        esl = bass.ts(ec, 512)
                        op = psum_op.tile([P, 512], f32, tag="op", name="op")
                        nc.tensor.matmul(
                            op, YT0[:, bass.ds(m0, P)], wo0_sb[:, esl],
                            start=True, stop=False,
                        )
                        nc.tensor.matmul(
                            op, YT1[:, bass.ds(m0, P)], wo1_sb[:, esl],
                            start=False, stop=True,
                        )
                        osb = out_pool.tile([P, 512], f32, name="osb")
                        psum_to_sbuf(osb, op)
                        nc.sync.dma_start(out_d[bass.ds(m0, P), esl], osb)

                def emit_outproj(b, jlo, jhi):
                    for j in range(jlo, jhi):
                        emit_outproj_tile(b, j)

                passes = [(b, h, pr) for b in range(B) for h in range(2) for pr in range(2)]
                pending = []
                filler = []  # (b, j) out-proj tiles interleaved as PE work
                for pi, (b, h, pr) in enumerate(passes):
                    dsl = bass.ds(D * h, D)
                    avs = [
                        psum_av.tile([D + 1, QCH], f32, tag="av", name="av")
                        for _ in range(2)
                    ]
                    for t in range(NKT):
                        ksl = bass.ds(b * S + t * P, P)
                        lhs_k = KT[dsl, ksl]
                        sc2 = psum_sc.tile([P, 2 * QCH], f32, tag="sc", name="sc2")
                        for qi in range(2):
                            qc = 2 * pr + qi
                            qsl = bass.ds(b * S + qc * QCH, QCH)
                            nc.tensor.matmul(
                                sc2[:, bass.ts(qi, QCH)], lhs_k, QT[dsl, qsl],
                                start=True, stop=True,
                            )
                        pt = pt_pool.tile([P, 2 * QCH], f16, tag="pt", name="pt")
                        bt = b * 16 + t
                        nc.scalar.activation(
                            pt, sc2, Exp, bias=mask_sb[:, bt : bt + 1], scale=1.0
                        )
                        lhs_v = Vtm[:, bt, h * (D + 1) : (h + 1) * (D + 1)]
                        for qi in range(2):
                            nc.tensor.matmul(
                                avs[qi], lhs_v, pt[:, bass.ts(qi, QCH)],
                                start=(t == 0), stop=(t == NKT - 1),
                            )
                        # interleave ready out-proj tiles as PE filler so the
                        # engine stays saturated through ACT hiccups
                        if filler and t % 3 == 2:
                            emit_outproj_tile(*filler.pop(0))
                    # stage accumulators to SBUF, freeing the PSUM banks
                    av_sbs = []
                    for qi in range(2):
                        av_sb = r_pool.tile(
                            [D + 1, QCH], f32, tag="avsb", bufs=6, name="avsb"
                        )
                        psum_to_sbuf(av_sb, avs[qi])
                        av_sbs.append(av_sb)
                    pending.append((b, h, pr, av_sbs))
                    if len(pending) > 1:
                        emit_norm(*pending.pop(0))
                    if pi == 4:
                        # norms for all of batch 0 have been emitted
                        filler.extend((0, j) for j in range(S // P))
                # drain leftover batch-0 filler, then the batch-1 tail
                for item in filler:
                    emit_outproj_tile(*item)
                emit_outproj(1, 0, S // (2 * P))
                emit_norm(*pending.pop(0))            # (1,1,1)
                emit_outproj(1, S // (2 * P), S // P)

    nc.compile()
    return nc


def kernel(x, mask, Wq, bq, Wk, bk, Wv, bv, Wo, bo):
    global LAST_RESULTS, _PROGRAM
    from concourse.bass_utils import run_bass_kernel_spmd

    if _PROGRAM is None:
        _PROGRAM = _build_program()
    nc = _PROGRAM

    f16 = np.float16
    x = np.asarray(x, dtype=np.float32)
    mask = np.asarray(mask)
    f32c = lambda a: np.ascontiguousarray(np.asarray(a, dtype=np.float32))

    xT = np.ascontiguousarray(x.reshape(M, E).T.astype(f16))     # [E, M]
    maskf = np.where(mask, np.float32(NEG), np.float32(0.0)).astype(np.float32)
    maskT = np.ascontiguousarray(
        maskf.reshape(B, 16, P).transpose(2, 0, 1).reshape(P, B * 16)
    )
    scale = np.float32(1.0 / np.sqrt(D))

    in_maps = []
    for c in range(NCORES):
        csl = slice(P * c, P * (c + 1))
        wq_c = (np.asarray(Wq, dtype=np.float32)[:, csl] * scale).astype(f16)
        wk_c = np.asarray(Wk, dtype=np.float32)[:, csl].astype(f16)
        wv_c = np.asarray(Wv, dtype=np.float32)[:, csl].astype(f16)
        in_maps.append(
            {
                "xT": xT,
                "wq": np.ascontiguousarray(wq_c.reshape(KC, P, P).transpose(1, 0, 2)),
                "wk": np.ascontiguousarray(wk_c.reshape(KC, P, P).transpose(1, 0, 2)),
                "wv": np.ascontiguousarray(wv_c.reshape(KC, P, P).transpose(1, 0, 2)),
                "wo0": np.ascontiguousarray(
                    np.asarray(Wo, dtype=np.float32)[P * c : P * c + D, :].astype(f16)
                ),
                "wo1": np.ascontiguousarray(
                    np.asarray(Wo, dtype=np.float32)[P * c + D : P * (c + 1), :].astype(f16)
                ),
                "bq": f32c(np.asarray(bq)[csl] * scale).reshape(P, 1),
                "bk": f32c(np.asarray(bk)[csl]).reshape(P, 1),
                "bv": f32c(np.asarray(bv)[csl]).reshape(P, 1),
                "maskT": maskT,
            }
        )

    trace = bool(os.environ.get("KERNEL_TRACE"))
    LAST_RESULTS = run_bass_kernel_spmd(
        nc, in_maps, list(range(NCORES)), trace=trace
    )

    acc = np.zeros((M, E), dtype=np.float64)
    for res in LAST_RESULTS.results:
        acc += res["out"].astype(np.float64)
    out = (acc + np.asarray(bo, dtype=np.float64)[None, :]).astype(np.float32)
    return out.reshape(B, S, E)


# revision 27
# speedup vs baseline: 2.0783x; 1.2094x over previous
"""Multi-head attention (B=2, S=2048, E=1024, H=16, D=64) on 8 TRN2 cores.

Sharding: tensor-parallel over heads. Core c owns heads {2c, 2c+1}:
  - Q/K/V projections column-sharded (128 cols each per core)
  - attention for the core's 2 heads (both batches)
  - out-projection row-sharded (128 rows of Wo) -> partial [4096,1024]
  - host sums the 8 partials and adds bo.

On-chip layout (everything "transposed"):
  - host passes xT [1024, 4096] (E-major, fp16) so the contraction dim
    lands on SBUF partitions with no on-device transpose of x
  - projections produce Q^T, K^T [128, 4096] (head-dim on partitions) and
    V^T, which is PE-transposed to token-major V tiles
  - scores are computed transposed: scores^T[kk, q] so softmax's key
    reduction can ride the attn@V matmul (ones-column in V) and the
    key-padding mask folds into the exp() per-partition bias
  - attn@V emits Y^T directly (head-dim on partitions), feeding the
    row-sharded out-projection without further transposes.

Perf notes:
  - matmul inputs fp16 (full PE rate); accumulation fp32 in PSUM;
    softmax normalization chain fp32
  - TRN2's PE p-state controller halves the clock when the engine idles,
    so the attention loop is shaped to keep PE saturated: exp() batched
    [128,1024] on ACT (faster per step than the PE work it feeds),
    normalization runs entirely on DVE+DMA (stride-0 partition-broadcast
    DMA instead of a ones-matmul), and each group's normalization is
    emitted one pass late so the PE never waits on the DVE reciprocal
  - consecutive PE matmuls share their stationary operand (weight-load
    amortization): key-tile-outer loops, paired m-chunks in projections
"""

import os
import numpy as np

B, S, E, H, D = 2, 2048, 1024, 16, 64
M = B * S            # 4096 tokens
P = 128              # partitions
NCORES = 8
KC = E // P          # 8 contraction chunks for projections
MCH = 512            # token chunk for projections
QCH = 512            # query chunk for attention
NQC = S // QCH       # 4 query chunks per batch
NKT = S // P         # 16 key tiles per batch
NEG = -1.0e30

LAST_RESULTS = None  # BassKernelResults of the most recent run (for test harness)
_PROGRAM = None


def _build_program():
    import concourse.bass as bass
    import concourse.tile as tile
    from concourse import bacc, mybir
    from concourse.masks import make_identity

    f32 = mybir.dt.float32
    f16 = mybir.dt.float16

    nc = bacc.Bacc(
        "TRN2",
        target_bir_lowering=False,
        debug=False,
        enable_asserts=False,
        num_devices=NCORES,
    )

    xT_d = nc.dram_tensor("xT", (E, M), f16, kind="ExternalInput").ap()
    wq_d = nc.dram_tensor("wq", (P, KC, P), f16, kind="ExternalInput").ap()
    wk_d = nc.dram_tensor("wk", (P, KC, P), f16, kind="ExternalInput").ap()
    wv_d = nc.dram_tensor("wv", (P, KC, P), f16, kind="ExternalInput").ap()
    wo0_d = nc.dram_tensor("wo0", (D, E), f16, kind="ExternalInput").ap()
    wo1_d = nc.dram_tensor("wo1", (D, E), f16, kind="ExternalInput").ap()
    bq_d = nc.dram_tensor("bq", (P, 1), f32, kind="ExternalInput").ap()
    bk_d = nc.dram_tensor("bk", (P, 1), f32, kind="ExternalInput").ap()
    bv_d = nc.dram_tensor("bv", (P, 1), f32, kind="ExternalInput").ap()
    maskT_d = nc.dram_tensor("maskT", (P, B * 16), f32, kind="ExternalInput").ap()
    out_d = nc.dram_tensor("out", (M, E), f32, kind="ExternalOutput").ap()
    rsc_d = nc.dram_tensor("rscratch", (16, QCH), f32, kind="Internal").ap()

    with tile.TileContext(nc) as tc:
        with (
            tc.tile_pool(name="consts", bufs=1) as consts,
            tc.tile_pool(name="big", bufs=1) as big,
            tc.tile_pool(name="xt_pool", bufs=6) as xt_pool,
            tc.tile_pool(name="vt_pool", bufs=2) as vt_pool,
            tc.tile_pool(name="pt_pool", bufs=6) as pt_pool,
            tc.tile_pool(name="r_pool", bufs=2) as r_pool,
            tc.tile_pool(name="out_pool", bufs=4) as out_pool,
        ):
            # ---- constants ----
            wq_sb = consts.tile([P, KC, P], f16)
            wk_sb = consts.tile([P, KC, P], f16)
            wv_sb = consts.tile([P, KC, P], f16)
            wo0_sb = consts.tile([D, E], f16)
            wo1_sb = consts.tile([D, E], f16)
            bq_sb = consts.tile([P, 1], f32)
            bk_sb = consts.tile([P, 1], f32)
            bv_sb = consts.tile([P, 1], f32)
            mask_sb = consts.tile([P, B * 16], f32)
            ident = consts.tile([P, P], f32)
            ones_h = consts.tile([P, M // P], f16)

            # constants go on the SWDGE queue so they don't block xt loads
            nc.gpsimd.dma_start(wq_sb, wq_d)
            nc.gpsimd.dma_start(wk_sb, wk_d)
            nc.gpsimd.dma_start(wv_sb, wv_d)
            nc.gpsimd.dma_start(wo0_sb, wo0_d)
            nc.gpsimd.dma_start(wo1_sb, wo1_d)
            nc.gpsimd.dma_start(bq_sb, bq_d)
            nc.gpsimd.dma_start(bk_sb, bk_d)
            nc.gpsimd.dma_start(bv_sb, bv_d)
            nc.gpsimd.dma_start(mask_sb, maskT_d)
            make_identity(nc, ident)
            nc.vector.memset(ones_h, 1.0)

            # ---- big persistent activations ----
            QT = big.tile([P, M], f16)       # Q^T: head-dims on partitions
            KT = big.tile([P, M], f16)
            # token-major V tiles: [tok, mt, 2*(64 cols + ones col)]
            Vtm = big.tile([P, M // P, 2 * (D + 1)], f16)
            YT0 = big.tile([D, M], f16)      # per-head attention output^T
            YT1 = big.tile([D, M], f16)

            ones_col = ones_h[:, 0 : M // P].rearrange("p (a b) -> p a b", b=1)
            nc.vector.tensor_copy(Vtm[:, :, D : D + 1], ones_col)
            nc.vector.tensor_copy(Vtm[:, :, 2 * D + 1 : 2 * D + 2], ones_col)

            # ---- phase 1: batch-0 projections (pairs share weight loads);
            # batch-1 projections are deferred into the attention passes ----
            with tc.tile_pool(name="psum_p1", bufs=6, space="PSUM") as psum_p1:
                for mcp in range(S // (2 * MCH)):
                    psums = []
                    for half in range(2):
                        mc = 2 * mcp + half
                        msl = bass.ts(mc, MCH)
                        qp = psum_p1.tile([P, MCH], f32, tag="p1", name="qp")
                        kp = psum_p1.tile([P, MCH], f32, tag="p1", name="kp")
                        vp = psum_p1.tile([P, MCH], f32, tag="p1", name="vp")
                        psums.append((msl, qp, kp, vp))
                    for kc in range(KC):
                        xts = []
                        for half in range(2):
                            msl = psums[half][0]
                            xt = xt_pool.tile([P, MCH], f16, tag="xt", name="xt")
                            nc.sync.dma_start(xt, xT_d[bass.ts(kc, P), msl])
                            xts.append(xt)
                        st, sp = kc == 0, kc == KC - 1
                        for wi, w_sb in ((1, wq_sb), (2, wk_sb), (3, wv_sb)):
                            for half in range(2):
                                nc.tensor.matmul(
                                    psums[half][wi], w_sb[:, kc, :], xts[half],
                                    start=st, stop=sp,
                                )
                    for half in range(2):
                        msl, qp, kp, vp = psums[half]
                        mc = 2 * mcp + half
                        nc.vector.tensor_scalar_add(QT[:, msl], qp, bq_sb)
                        nc.vector.tensor_scalar_add(KT[:, msl], kp, bk_sb)
                        vt = vt_pool.tile([P, MCH], f32, name="vt")
                        nc.vector.tensor_scalar_add(vt, vp, bv_sb)
                        for j in range(MCH // P):
                            mt = mc * (MCH // P) + j
                            vtp = psum_p1.tile([P, P], f32, tag="vtp", bufs=2, name="vtp")
                            nc.tensor.transpose(vtp, vt[:, bass.ts(j, P)], ident)
                            nc.vector.tensor_copy(Vtm[:, mt, 0:D], vtp[:, 0:D])
                            nc.vector.tensor_copy(
                                Vtm[:, mt, D + 1 : 2 * D + 1], vtp[:, D : 2 * D]
                            )

            # ---- phase 2: attention, deferred normalization, out-proj ----
            Exp = mybir.ActivationFunctionType.Exp
            with (
                tc.tile_pool(name="psum_sc", bufs=2, space="PSUM") as psum_sc,
                tc.tile_pool(name="psum_av", bufs=2, space="PSUM") as psum_av,
                tc.tile_pool(name="psum_op", bufs=2, space="PSUM") as psum_op,
            ):
                norm_idx = [0]

                def psum_to_sbuf(dst, src):
                    # DVE only: ACT must stay a pure-exp stream, or its stalls
                    # starve the PE and drop the p-state
                    nc.vector.tensor_copy(dst, src)

                def emit_norm(b, h, pr, av_sbs):
                    YT = YT0 if h == 0 else YT1
                    for qi in range(2):
                        qc = 2 * pr + qi
                        qsl = bass.ds(b * S + qc * QCH, QCH)
                        av_sb = av_sbs[qi]
                        # partition-broadcast the raw sums [1,512] -> [64,512]
                        # via DRAM bounce (SBUF-source DMAs cannot have a zero
                        # partition step), then reciprocal at base partition 0
                        # (custom-DVE approx ops misbehave at base 64)
                        ni = norm_idx[0]
                        norm_idx[0] += 1
                        nc.sync.dma_start(rsc_d[ni, :], av_sb[D : D + 1, :])
                        sb = r_pool.tile([D, QCH], f32, tag="sb", bufs=4, name="sb")
                        src = rsc_d[ni : ni + 1, :]
                        src_b = bass.AP(
                            tensor=src.tensor,
                            offset=src.offset,
                            ap=[[0, D]] + [list(x) for x in src.ap[1:]],
                        )
                        nc.sync.dma_start(sb, src_b)
                        rbs = r_pool.tile([D, QCH], f32, tag="rbs", bufs=4, name="rbs")
                        rsc2 = r_pool.tile([D, QCH], f32, tag="rsc2", name="rsc2")
                        nc.vector.reciprocal_approx_accurate(rbs, sb, rsc2)
                        nc.vector.tensor_mul(YT[:, qsl], av_sb[0:D, :], rbs)

                def emit_outproj_tile(b, j):
                    m0 = b * S + j * P
                    for ec in range(E // 512):
                        esl = bass.ts(ec, 512)
                        op = psum_op.tile([P, 512], f32, tag="op", name="op")
                        nc.tensor.matmul(
                            op, YT0[:, bass.ds(m0, P)], wo0_sb[:, esl],
                            start=True, stop=False,
                        )
                        nc.tensor.matmul(
                            op, YT1[:, bass.ds(m0, P)], wo1_sb[:, esl],
                            start=False, stop=True,
                        )
                        osb = out_pool.tile([P, 512], f32, name="osb")
                        psum_to_sbuf(osb, op)
                        nc.sync.dma_start(out_d[bass.ds(m0, P), esl], osb)

                def emit_outproj(b, jlo, jhi):
                    for j in range(jlo, jhi):
                        emit_outproj_tile(b, j)

                # --- deferred batch-1 projection filler units ---
                def make_proj_units():
                    units = []
                    for mc in range(S // MCH, M // MCH):
                        msl = bass.ts(mc, MCH)
                        state = {}

                        def u_q(mc=mc, msl=msl, state=state):
                            xts = []
                            for kc in range(KC):
                                xt = xt_pool.tile(
                                    [P, MCH], f16, tag="xt2", bufs=18, name="xt2"
                                )
                                nc.sync.dma_start(xt, xT_d[bass.ts(kc, P), msl])
                                xts.append(xt)
                            state["xts"] = xts
                            qp = psum_op.tile([P, MCH], f32, tag="op", name="qp2")
                            for kc in range(KC):
                                nc.tensor.matmul(
                                    qp, wq_sb[:, kc, :], xts[kc],
                                    start=(kc == 0), stop=(kc == KC - 1),
                                )
                            nc.vector.tensor_scalar_add(QT[:, msl], qp, bq_sb)

                        def u_k(mc=mc, msl=msl, state=state):
                            kp = psum_op.tile([P, MCH], f32, tag="op", name="kp2")
                            for kc in range(KC):
                                nc.tensor.matmul(
                                    kp, wk_sb[:, kc, :], state["xts"][kc],
                                    start=(kc == 0), stop=(kc == KC - 1),
                                )
                            nc.vector.tensor_scalar_add(KT[:, msl], kp, bk_sb)

                        def u_v(mc=mc, msl=msl, state=state):
                            vp = psum_op.tile([P, MCH], f32, tag="op", name="vp2")
                            for kc in range(KC):
                                nc.tensor.matmul(
                                    vp, wv_sb[:, kc, :], state["xts"][kc],
                                    start=(kc == 0), stop=(kc == KC - 1),
                                )
                            vt = vt_pool.tile([P, MCH], f32, name="vt2", tag="vt2")
                            nc.vector.tensor_scalar_add(vt, vp, bv_sb)
                            state["vt"] = vt

                        def u_t(mc=mc, state=state):
                            vt = state["vt"]
                            for j in range(MCH // P):
                                mt = mc * (MCH // P) + j
                                vtp = psum_op.tile(
                                    [P, P], f32, tag="op", name="vtp2"
                                )
                                nc.tensor.transpose(vtp, vt[:, bass.ts(j, P)], ident)
                                nc.vector.tensor_copy(Vtm[:, mt, 0:D], vtp[:, 0:D])
                                nc.vector.tensor_copy(
                                    Vtm[:, mt, D + 1 : 2 * D + 1], vtp[:, D : 2 * D]
                                )

                        units += [u_q, u_k, u_v, u_t]
                    return units

                passes = [(b, h, pr) for b in range(B) for h in range(2) for pr in range(2)]
                pending = []
                filler = list(make_proj_units())
                for pi, (b, h, pr) in enumerate(passes):
                    if pi == 4:
                        # deferred projections must be fully emitted before
                        # any batch-1 read (emission order defines dataflow)
                        while filler:
                            filler.pop(0)()
                        filler = [
                            (lambda b0=0, j0=j: emit_outproj_tile(b0, j0))
                            for j in range(S // P)
                        ]
                    dsl = bass.ds(D * h, D)
                    avs = [
                        psum_av.tile([D + 1, QCH], f32, tag="av", name="av")
                        for _ in range(2)
                    ]
                    for t in range(NKT):
                        ksl = bass.ds(b * S + t * P, P)
                        lhs_k = KT[dsl, ksl]
                        sc2 = psum_sc.tile([P, 2 * QCH], f32, tag="sc", name="sc2")
                        for qi in range(2):
                            qc = 2 * pr + qi
                            qsl = bass.ds(b * S + qc * QCH, QCH)
                            nc.tensor.matmul(
                                sc2[:, bass.ts(qi, QCH)], lhs_k, QT[dsl, qsl],
                                start=True, stop=True,
                            )
                        pt = pt_pool.tile([P, 2 * QCH], f16, tag="pt", name="pt")
                        bt = b * 16 + t
                        nc.scalar.activation(
                            pt, sc2, Exp, bias=mask_sb[:, bt : bt + 1], scale=1.0
                        )
                        lhs_v = Vtm[:, bt, h * (D + 1) : (h + 1) * (D + 1)]
                        for qi in range(2):
                            nc.tensor.matmul(
                                avs[qi], lhs_v, pt[:, bass.ts(qi, QCH)],
                                start=(t == 0), stop=(t == NKT - 1),
                            )
                        # early-emit the previous pass's normalization (DVE/DMA
                        # only) so its reciprocal never gates later PE work
                        if t == 1 and pending:
                            emit_norm(*pending.pop(0))
                            if pi == 7:
                                # batch-1 pair-0 columns are now normalized
                                filler.extend(
                                    (lambda b1=1, j1=j: emit_outproj_tile(b1, j1))
                                    for j in range(S // (2 * P))
                                )
                        # interleave independent PE work (deferred projections,
                        # ready out-proj tiles) to keep the PE saturated
                        if filler and t % 3 == 2:
                            filler.pop(0)()
                    # stage accumulators to SBUF, freeing the PSUM banks
                    av_sbs = []
                    for qi in range(2):
                        av_sb = r_pool.tile(
                            [D + 1, QCH], f32, tag="avsb", bufs=6, name="avsb"
                        )
                        psum_to_sbuf(av_sb, avs[qi])
                        av_sbs.append(av_sb)
                    pending.append((b, h, pr, av_sbs))
                # tail: drain remaining filler, last norm, batch-1 pair-1
                while filler:
                    filler.pop(0)()
                emit_norm(*pending.pop(0))            # (1,1,1)
                emit_outproj(1, S // (2 * P), S // P)

    nc.compile()
    return nc


def kernel(x, mask, Wq, bq, Wk, bk, Wv, bv, Wo, bo):
    global LAST_RESULTS, _PROGRAM
    from concourse.bass_utils import run_bass_kernel_spmd

    if _PROGRAM is None:
        _PROGRAM = _build_program()
    nc = _PROGRAM

    f16 = np.float16
    x = np.asarray(x, dtype=np.float32)
    mask = np.asarray(mask)
    f32c = lambda a: np.ascontiguousarray(np.asarray(a, dtype=np.float32))

    xT = np.ascontiguousarray(x.reshape(M, E).T.astype(f16))     # [E, M]
    maskf = np.where(mask, np.float32(NEG), np.float32(0.0)).astype(np.float32)
    maskT = np.ascontiguousarray(
        maskf.reshape(B, 16, P).transpose(2, 0, 1).reshape(P, B * 16)
    )
    scale = np.float32(1.0 / np.sqrt(D))

    in_maps = []
    for c in range(NCORES):
        csl = slice(P * c, P * (c + 1))
        wq_c = (np.asarray(Wq, dtype=np.float32)[:, csl] * scale).astype(f16)
        wk_c = np.asarray(Wk, dtype=np.float32)[:, csl].astype(f16)
        wv_c = np.asarray(Wv, dtype=np.float32)[:, csl].astype(f16)
        in_maps.append(
            {
                "xT": xT,
                "wq": np.ascontiguousarray(wq_c.reshape(KC, P, P).transpose(1, 0, 2)),
                "wk": np.ascontiguousarray(wk_c.reshape(KC, P, P).transpose(1, 0, 2)),
                "wv": np.ascontiguousarray(wv_c.reshape(KC, P, P).transpose(1, 0, 2)),
                "wo0": np.ascontiguousarray(
                    np.asarray(Wo, dtype=np.float32)[P * c : P * c + D, :].astype(f16)
                ),
                "wo1": np.ascontiguousarray(
                    np.asarray(Wo, dtype=np.float32)[P * c + D : P * (c + 1), :].astype(f16)
                ),
                "bq": f32c(np.asarray(bq)[csl] * scale).reshape(P, 1),
                "bk": f32c(np.asarray(bk)[csl]).reshape(P, 1),
                "bv": f32c(np.asarray(bv)[csl]).reshape(P, 1),
                "maskT": maskT,
            }
        )

    trace = bool(os.environ.get("KERNEL_TRACE"))
    LAST_RESULTS = run_bass_kernel_spmd(
        nc, in_maps, list(range(NCORES)), trace=trace
    )

    acc = np.zeros((M, E), dtype=np.float64)
    for res in LAST_RESULTS.results:
        acc += res["out"].astype(np.float64)
    out = (acc + np.asarray(bo, dtype=np.float64)[None, :]).astype(np.float32)
    return out.reshape(B, S, E)


# revision 30
# speedup vs baseline: 2.1021x; 1.0114x over previous
"""Multi-head attention (B=2, S=2048, E=1024, H=16, D=64) on 8 TRN2 cores.

Sharding: tensor-parallel over heads. Core c owns heads {2c, 2c+1}:
  - Q/K/V projections column-sharded (128 cols each per core)
  - attention for the core's 2 heads (both batches)
  - out-projection row-sharded (128 rows of Wo) -> partial [4096,1024]
  - host sums the 8 partials and adds bo.

On-chip layout (everything "transposed"):
  - host passes xT [1024, 4096] (E-major, fp16) so the contraction dim
    lands on SBUF partitions with no on-device transpose of x
  - projections produce Q^T, K^T [128, 4096] (head-dim on partitions) and
    V^T, which is PE-transposed to token-major V tiles
  - scores are computed transposed: scores^T[kk, q] so softmax's key
    reduction can ride the attn@V matmul (ones-column in V) and the
    key-padding mask folds into the exp() per-partition bias
  - attn@V emits Y^T directly (head-dim on partitions), feeding the
    row-sharded out-projection without further transposes.

Perf notes:
  - matmul inputs fp16 (full PE rate); accumulation fp32 in PSUM;
    softmax normalization chain fp32
  - TRN2's PE p-state controller halves the clock when the engine idles,
    so the attention loop is shaped to keep PE saturated: exp() batched
    [128,1024] on ACT (faster per step than the PE work it feeds),
    normalization runs entirely on DVE+DMA (stride-0 partition-broadcast
    DMA instead of a ones-matmul), and each group's normalization is
    emitted one pass late so the PE never waits on the DVE reciprocal
  - consecutive PE matmuls share their stationary operand (weight-load
    amortization): key-tile-outer loops, paired m-chunks in projections
"""

import os
import numpy as np

B, S, E, H, D = 2, 2048, 1024, 16, 64
M = B * S            # 4096 tokens
P = 128              # partitions
NCORES = 8
KC = E // P          # 8 contraction chunks for projections
MCH = 512            # token chunk for projections
QCH = 512            # query chunk for attention
NQC = S // QCH       # 4 query chunks per batch
NKT = S // P         # 16 key tiles per batch
NEG = -1.0e30

LAST_RESULTS = None  # BassKernelResults of the most recent run (for test harness)
_PROGRAM = None


def _build_program():
    import concourse.bass as bass
    import concourse.tile as tile
    from concourse import bacc, mybir
    from concourse.masks import make_identity

    f32 = mybir.dt.float32
    f16 = mybir.dt.float16

    nc = bacc.Bacc(
        "TRN2",
        target_bir_lowering=False,
        debug=False,
        enable_asserts=False,
        num_devices=NCORES,
    )

    xT_d = nc.dram_tensor("xT", (E, M), f16, kind="ExternalInput").ap()
    wq_d = nc.dram_tensor("wq", (P, KC, P), f16, kind="ExternalInput").ap()
    wk_d = nc.dram_tensor("wk", (P, KC, P), f16, kind="ExternalInput").ap()
    wv_d = nc.dram_tensor("wv", (P, KC, P), f16, kind="ExternalInput").ap()
    wo0_d = nc.dram_tensor("wo0", (D, E), f16, kind="ExternalInput").ap()
    wo1_d = nc.dram_tensor("wo1", (D, E), f16, kind="ExternalInput").ap()
    bq_d = nc.dram_tensor("bq", (P, 1), f32, kind="ExternalInput").ap()
    bk_d = nc.dram_tensor("bk", (P, 1), f32, kind="ExternalInput").ap()
    bv_d = nc.dram_tensor("bv", (P, 1), f32, kind="ExternalInput").ap()
    maskT_d = nc.dram_tensor("maskT", (P, B * 16), f32, kind="ExternalInput").ap()
    out_d = nc.dram_tensor("out", (M, E), f32, kind="ExternalOutput").ap()
    rsc_d = nc.dram_tensor("rscratch", (16, QCH), f32, kind="Internal").ap()

    with tile.TileContext(nc) as tc:
        with (
            tc.tile_pool(name="consts", bufs=1) as consts,
            tc.tile_pool(name="big", bufs=1) as big,
            tc.tile_pool(name="xt_pool", bufs=6) as xt_pool,
            tc.tile_pool(name="vt_pool", bufs=2) as vt_pool,
            tc.tile_pool(name="pt_pool", bufs=6) as pt_pool,
            tc.tile_pool(name="r_pool", bufs=2) as r_pool,
            tc.tile_pool(name="out_pool", bufs=4) as out_pool,
        ):
            # ---- constants ----
            wq_sb = consts.tile([P, KC, P], f16)
            wk_sb = consts.tile([P, KC, P], f16)
            wv_sb = consts.tile([P, KC, P], f16)
            wo0_sb = consts.tile([D, E], f16)
            wo1_sb = consts.tile([D, E], f16)
            bq_sb = consts.tile([P, 1], f32)
            bk_sb = consts.tile([P, 1], f32)
            bv_sb = consts.tile([P, 1], f32)
            mask_sb = consts.tile([P, B * 16], f32)
            ident = consts.tile([P, P], f32)
            ones_h = consts.tile([P, M // P], f16)

            # constants go on the SWDGE queue so they don't block xt loads
            nc.gpsimd.dma_start(wq_sb, wq_d)
            nc.gpsimd.dma_start(wk_sb, wk_d)
            nc.gpsimd.dma_start(wv_sb, wv_d)
            nc.gpsimd.dma_start(wo0_sb, wo0_d)
            nc.gpsimd.dma_start(wo1_sb, wo1_d)
            nc.gpsimd.dma_start(bq_sb, bq_d)
            nc.gpsimd.dma_start(bk_sb, bk_d)
            nc.gpsimd.dma_start(bv_sb, bv_d)
            nc.gpsimd.dma_start(mask_sb, maskT_d)
            make_identity(nc, ident)
            nc.vector.memset(ones_h, 1.0)

            # ---- big persistent activations ----
            QT = big.tile([P, M], f16)       # Q^T: head-dims on partitions
            KT = big.tile([P, M], f16)
            # token-major V tiles: [tok, mt, 2*(64 cols + ones col)]
            Vtm = big.tile([P, M // P, 2 * (D + 1)], f16)
            YT0 = big.tile([D, M], f16)      # per-head attention output^T
            YT1 = big.tile([D, M], f16)

            ones_col = ones_h[:, 0 : M // P].rearrange("p (a b) -> p a b", b=1)
            nc.vector.tensor_copy(Vtm[:, :, D : D + 1], ones_col)
            nc.vector.tensor_copy(Vtm[:, :, 2 * D + 1 : 2 * D + 2], ones_col)

            # ---- phase 1: batch-0 projections (pairs share weight loads);
            # batch-1 projections are deferred into the attention passes ----
            with tc.tile_pool(name="psum_p1", bufs=6, space="PSUM") as psum_p1:
                for mcp in range(S // (2 * MCH)):
                    psums = []
                    for half in range(2):
                        mc = 2 * mcp + half
                        msl = bass.ts(mc, MCH)
                        qp = psum_p1.tile([P, MCH], f32, tag="p1", name="qp")
                        kp = psum_p1.tile([P, MCH], f32, tag="p1", name="kp")
                        vp = psum_p1.tile([P, MCH], f32, tag="p1", name="vp")
                        psums.append((msl, qp, kp, vp))
                    for kc in range(KC):
                        xts = []
                        for half in range(2):
                            msl = psums[half][0]
                            xt = xt_pool.tile([P, MCH], f16, tag="xt", name="xt")
                            nc.sync.dma_start(xt, xT_d[bass.ts(kc, P), msl])
                            xts.append(xt)
                        st, sp = kc == 0, kc == KC - 1
                        for wi, w_sb in ((1, wq_sb), (2, wk_sb), (3, wv_sb)):
                            for half in range(2):
                                nc.tensor.matmul(
                                    psums[half][wi], w_sb[:, kc, :], xts[half],
                                    start=st, stop=sp,
                                )
                    for half in range(2):
                        msl, qp, kp, vp = psums[half]
                        mc = 2 * mcp + half
                        nc.vector.tensor_scalar_add(QT[:, msl], qp, bq_sb)
                        nc.vector.tensor_scalar_add(KT[:, msl], kp, bk_sb)
                        vt = vt_pool.tile([P, MCH], f32, name="vt")
                        nc.vector.tensor_scalar_add(vt, vp, bv_sb)
                        for j in range(MCH // P):
                            mt = mc * (MCH // P) + j
                            vtp = psum_p1.tile([P, P], f32, tag="vtp", bufs=2, name="vtp")
                            nc.tensor.transpose(vtp, vt[:, bass.ts(j, P)], ident)
                            nc.vector.tensor_copy(Vtm[:, mt, 0:D], vtp[:, 0:D])
                            nc.vector.tensor_copy(
                                Vtm[:, mt, D + 1 : 2 * D + 1], vtp[:, D : 2 * D]
                            )

            # ---- phase 2: attention, deferred normalization, out-proj ----
            Exp = mybir.ActivationFunctionType.Exp
            with (
                tc.tile_pool(name="psum_sc", bufs=2, space="PSUM") as psum_sc,
                tc.tile_pool(name="psum_av", bufs=2, space="PSUM") as psum_av,
                tc.tile_pool(name="psum_op", bufs=2, space="PSUM") as psum_op,
            ):
                norm_idx = [0]

                def psum_to_sbuf(dst, src):
                    # DVE only: ACT must stay a pure-exp stream, or its stalls
                    # starve the PE and drop the p-state
                    nc.vector.tensor_copy(dst, src)

                def emit_norm_qc(b, h, pr, qi, av_sb):
                    YT = YT0 if h == 0 else YT1
                    if True:
                        qc = 2 * pr + qi
                        qsl = bass.ds(b * S + qc * QCH, QCH)
                        # partition-broadcast the raw sums [1,512] -> [64,512]
                        # via DRAM bounce (SBUF-source DMAs cannot have a zero
                        # partition step), then reciprocal at base partition 0
                        # (custom-DVE approx ops misbehave at base 64)
                        ni = norm_idx[0]
                        norm_idx[0] += 1
                        nc.sync.dma_start(rsc_d[ni, :], av_sb[D : D + 1, :])
                        sb = r_pool.tile([D, QCH], f32, tag="sb", bufs=4, name="sb")
                        src = rsc_d[ni : ni + 1, :]
                        src_b = bass.AP(
                            tensor=src.tensor,
                            offset=src.offset,
                            ap=[[0, D]] + [list(x) for x in src.ap[1:]],
                        )
                        nc.sync.dma_start(sb, src_b)
                        rbs = r_pool.tile([D, QCH], f32, tag="rbs", bufs=4, name="rbs")
                        rsc2 = r_pool.tile([D, QCH], f32, tag="rsc2", name="rsc2")
                        nc.vector.reciprocal_approx_accurate(rbs, sb, rsc2)
                        nc.vector.tensor_mul(YT[:, qsl], av_sb[0:D, :], rbs)

                def emit_norm(b, h, pr, av_sbs):
                    for qi in range(2):
                        emit_norm_qc(b, h, pr, qi, av_sbs[qi])

                def emit_outproj_tile(b, j):
                    m0 = b * S + j * P
                    for ec in range(E // 512):
                        esl = bass.ts(ec, 512)
                        op = psum_op.tile([P, 512], f32, tag="op", name="op")
                        nc.tensor.matmul(
                            op, YT0[:, bass.ds(m0, P)], wo0_sb[:, esl],
                            start=True, stop=False,
                        )
                        nc.tensor.matmul(
                            op, YT1[:, bass.ds(m0, P)], wo1_sb[:, esl],
                            start=False, stop=True,
                        )
                        osb = out_pool.tile([P, 512], f32, name="osb")
                        psum_to_sbuf(osb, op)
                        nc.sync.dma_start(out_d[bass.ds(m0, P), esl], osb)

                def emit_outproj(b, jlo, jhi):
                    for j in range(jlo, jhi):
                        emit_outproj_tile(b, j)

                # --- deferred batch-1 projection filler units ---
                def make_proj_units():
                    units = []
                    for mc in range(S // MCH, M // MCH):
                        msl = bass.ts(mc, MCH)
                        state = {}

                        def u_q(mc=mc, msl=msl, state=state):
                            xts = []
                            for kc in range(KC):
                                xt = xt_pool.tile(
                                    [P, MCH], f16, tag="xt2", bufs=18, name="xt2"
                                )
                                nc.sync.dma_start(xt, xT_d[bass.ts(kc, P), msl])
                                xts.append(xt)
                            state["xts"] = xts
                            qp = psum_op.tile([P, MCH], f32, tag="op", name="qp2")
                            for kc in range(KC):
                                nc.tensor.matmul(
                                    qp, wq_sb[:, kc, :], xts[kc],
                                    start=(kc == 0), stop=(kc == KC - 1),
                                )
                            nc.vector.tensor_scalar_add(QT[:, msl], qp, bq_sb)

                        def u_k(mc=mc, msl=msl, state=state):
                            kp = psum_op.tile([P, MCH], f32, tag="op", name="kp2")
                            for kc in range(KC):
                                nc.tensor.matmul(
                                    kp, wk_sb[:, kc, :], state["xts"][kc],
                                    start=(kc == 0), stop=(kc == KC - 1),
                                )
                            nc.vector.tensor_scalar_add(KT[:, msl], kp, bk_sb)

                        def u_v(mc=mc, msl=msl, state=state):
                            vp = psum_op.tile([P, MCH], f32, tag="op", name="vp2")
                            for kc in range(KC):
                                nc.tensor.matmul(
                                    vp, wv_sb[:, kc, :], state["xts"][kc],
                                    start=(kc == 0), stop=(kc == KC - 1),
                                )
                            vt = vt_pool.tile([P, MCH], f32, name="vt2", tag="vt2")
                            nc.vector.tensor_scalar_add(vt, vp, bv_sb)
                            state["vt"] = vt

                        def u_t(mc=mc, state=state):
                            vt = state["vt"]
                            for j in range(MCH // P):
                                mt = mc * (MCH // P) + j
                                vtp = psum_op.tile(
                                    [P, P], f32, tag="op", name="vtp2"
                                )
                                nc.tensor.transpose(vtp, vt[:, bass.ts(j, P)], ident)
                                nc.vector.tensor_copy(Vtm[:, mt, 0:D], vtp[:, 0:D])
                                nc.vector.tensor_copy(
                                    Vtm[:, mt, D + 1 : 2 * D + 1], vtp[:, D : 2 * D]
                                )

                        units += [u_q, u_k, u_v, u_t]
                    return units

                passes = [(b, h, pr) for b in range(B) for h in range(2) for pr in range(2)]
                pending = []
                filler = list(make_proj_units())
                for pi, (b, h, pr) in enumerate(passes):
                    if pi == 4:
                        # deferred projections must be fully emitted before
                        # any batch-1 read (emission order defines dataflow)
                        while filler:
                            filler.pop(0)()
                        filler = [
                            (lambda b0=0, j0=j: emit_outproj_tile(b0, j0))
                            for j in range(S // P)
                        ]
                    dsl = bass.ds(D * h, D)
                    avs = [
                        psum_av.tile([D + 1, QCH], f32, tag="av", name="av")
                        for _ in range(2)
                    ]
                    for t in range(NKT):
                        ksl = bass.ds(b * S + t * P, P)
                        lhs_k = KT[dsl, ksl]
                        sc2 = psum_sc.tile([P, 2 * QCH], f32, tag="sc", name="sc2")
                        for qi in range(2):
                            qc = 2 * pr + qi
                            qsl = bass.ds(b * S + qc * QCH, QCH)
                            nc.tensor.matmul(
                                sc2[:, bass.ts(qi, QCH)], lhs_k, QT[dsl, qsl],
                                start=True, stop=True,
                            )
                        pt = pt_pool.tile([P, 2 * QCH], f16, tag="pt", name="pt")
                        bt = b * 16 + t
                        nc.scalar.activation(
                            pt, sc2, Exp, bias=mask_sb[:, bt : bt + 1], scale=1.0
                        )
                        lhs_v = Vtm[:, bt, h * (D + 1) : (h + 1) * (D + 1)]
                        for qi in range(2):
                            nc.tensor.matmul(
                                avs[qi], lhs_v, pt[:, bass.ts(qi, QCH)],
                                start=(t == 0), stop=(t == NKT - 1),
                            )
                        # early-emit the previous pass's normalization (DVE/DMA
                        # only) so its reciprocal never gates later PE work
                        if t == 1 and pending:
                            emit_norm(*pending.pop(0))
                            if pi == 7:
                                # batch-1 pair-0 columns are now normalized
                                filler.extend(
                                    (lambda b1=1, j1=j: emit_outproj_tile(b1, j1))
                                    for j in range(S // (2 * P))
                                )
                        # interleave independent PE work (deferred projections,
                        # ready out-proj tiles) to keep the PE saturated
                        if filler and t % 3 == 2:
                            filler.pop(0)()
                    # stage accumulators to SBUF, freeing the PSUM banks
                    av_sbs = []
                    for qi in range(2):
                        av_sb = r_pool.tile(
                            [D + 1, QCH], f32, tag="avsb", bufs=6, name="avsb"
                        )
                        psum_to_sbuf(av_sb, avs[qi])
                        av_sbs.append(av_sb)
                    pending.append((b, h, pr, av_sbs))
                # tail: drain remaining filler; interleave the last norm
                # per-query-chunk with the out-proj tiles it unblocks
                b_l, h_l, pr_l, av_sbs_l = pending.pop(0)   # (1,1,1)
                emit_norm_qc(b_l, h_l, pr_l, 0, av_sbs_l[0])
                while filler:
                    filler.pop(0)()
                emit_outproj(1, 2 * S // (4 * P), 3 * S // (4 * P))   # qc2 tokens
                emit_norm_qc(b_l, h_l, pr_l, 1, av_sbs_l[1])
                emit_outproj(1, 3 * S // (4 * P), S // P)             # qc3 tokens

    nc.compile()
    return nc


def kernel(x, mask, Wq, bq, Wk, bk, Wv, bv, Wo, bo):
    global LAST_RESULTS, _PROGRAM
    from concourse.bass_utils import run_bass_kernel_spmd

    if _PROGRAM is None:
        _PROGRAM = _build_program()
    nc = _PROGRAM

    f16 = np.float16
    x = np.asarray(x, dtype=np.float32)
    mask = np.asarray(mask)
    f32c = lambda a: np.ascontiguousarray(np.asarray(a, dtype=np.float32))

    xT = np.ascontiguousarray(x.reshape(M, E).T.astype(f16))     # [E, M]
    maskf = np.where(mask, np.float32(NEG), np.float32(0.0)).astype(np.float32)
    maskT = np.ascontiguousarray(
        maskf.reshape(B, 16, P).transpose(2, 0, 1).reshape(P, B * 16)
    )
    scale = np.float32(1.0 / np.sqrt(D))

    in_maps = []
    for c in range(NCORES):
        csl = slice(P * c, P * (c + 1))
        wq_c = (np.asarray(Wq, dtype=np.float32)[:, csl] * scale).astype(f16)
        wk_c = np.asarray(Wk, dtype=np.float32)[:, csl].astype(f16)
        wv_c = np.asarray(Wv, dtype=np.float32)[:, csl].astype(f16)
        in_maps.append(
            {
                "xT": xT,
                "wq": np.ascontiguousarray(wq_c.reshape(KC, P, P).transpose(1, 0, 2)),
                "wk": np.ascontiguousarray(wk_c.reshape(KC, P, P).transpose(1, 0, 2)),
                "wv": np.ascontiguousarray(wv_c.reshape(KC, P, P).transpose(1, 0, 2)),
                "wo0": np.ascontiguousarray(
                    np.asarray(Wo, dtype=np.float32)[P * c : P * c + D, :].astype(f16)
                ),
                "wo1": np.ascontiguousarray(
                    np.asarray(Wo, dtype=np.float32)[P * c + D : P * (c + 1), :].astype(f16)
                ),
                "bq": f32c(np.asarray(bq)[csl] * scale).reshape(P, 1),
                "bk": f32c(np.asarray(bk)[csl]).reshape(P, 1),
                "bv": f32c(np.asarray(bv)[csl]).reshape(P, 1),
                "maskT": maskT,
            }
        )

    trace = bool(os.environ.get("KERNEL_TRACE"))
    LAST_RESULTS = run_bass_kernel_spmd(
        nc, in_maps, list(range(NCORES)), trace=trace
    )

    acc = np.zeros((M, E), dtype=np.float64)
    for res in LAST_RESULTS.results:
        acc += res["out"].astype(np.float64)
    out = (acc + np.asarray(bo, dtype=np.float64)[None, :]).astype(np.float32)
    return out.reshape(B, S, E)
